# revision 42
# baseline (speedup 1.0000x reference)
"""Trainium2 Bass kernel for nn_Attention_41704132444382.

Masked-linear QKV projection + 16-head attention + masked-linear output
projection, tensor-parallel over heads across 8 NeuronCores (2 heads/core).

Design (ScalarE exp streaming ~128us is the roofline; PE ~ matches it):
  - Host: gates both masked-linear weights (sigmoid(m)>0.5), transposes x,
    casts x / wqkv / wo to bf16 (wqkv/wo values are +-c, near-exact in bf16).
  - QKV: xq bf16 tiles; lhsT = gated wqkv bf16 (FWL weight loads); psum
    [128,512] chains; q is stored split-precision (hi/lo bf16 pair) so the
    S matmul recovers fp32-exact q in the otherwise-idle half of the PE
    array (k is duplicated); k/q evacuated by ScalarE, v by DVE.  V^T is
    PE-transposed (bf16) to v1/v2 [t, dv|1] tiles whose ones column makes
    the PV matmul emit the softmax denominator for free (M=65).
  - The first attention block's S/exp overlaps the second half of QKV
    (its PV is deferred through a deep e-ring; QKV runs on 2 ping-pong
    psum banks during the overlap so s+qkv+vt fit in 8 banks).
  - Attention per 1024-query block, h-offset pipeline: per key-tile jt,
    one 1024-wide exp ACT per head (scale=1/32) -> e_h bf16; PV lags via
    a pend-queue drained to per-jt depth targets so block-boundary work
    (norm broadcast + out-projection) flows through the pv psum rings
    without stalling ScalarE.
  - Softmax denominators: pv row 64 -> [1,1024], PE-transpose chunks to
    partitions, DVE reciprocal, PE-transpose back, DMA row-gather, then
    a K=1 ones-matmul broadcast and one normalize tensor_tensor per head.
  - Output projection: lhsT = attnT bf16 (FWL), po pairs [128,1024] in
    the pv psum rings, DVE evac (split with ScalarE in the tail), DMA out.
"""

import os
import sys

import numpy as np

sys.path.insert(0, "/opt/trn_rl_repo")

import concourse.bass as bass
import concourse.mybir as mybir
from concourse import bacc
from concourse.masks import make_identity
from concourse.tile import TileContext

DIM = 1024
HEADS = 16
B = 2
N = 2048
T = B * N  # 4096 flattened tokens
NCORES = 8
HPC = HEADS // NCORES  # 2 heads per core
DV = HPC * 64  # 128 head-dims per core
SCALE = DIM ** (-0.5)  # 1/32

F32 = mybir.dt.float32
F32R = mybir.dt.float32r
BF16 = mybir.dt.bfloat16

Copy = mybir.ActivationFunctionType.Copy
Exp = mybir.ActivationFunctionType.Exp
mult = mybir.AluOpType.mult


def build_nc():
    nc = bacc.Bacc("TRN2", target_bir_lowering=True)
    xT_d = nc.declare_dram_parameter("xT", [DIM, T], BF16, isOutput=False)
    wqkvT_d = nc.declare_dram_parameter("wqkvT", [DIM, 384], BF16, isOutput=False)
    woT_d = nc.declare_dram_parameter("woT", [DV, DIM], BF16, isOutput=False)
    out_d = nc.declare_dram_parameter("out", [T, DIM], F32, isOutput=True)

    with TileContext(nc) as tc:
        with tc.tile_pool(name="persist", bufs=1) as pp:
            qs = [pp.tile([128, T], BF16, name=f"qs{h}") for h in range(HPC)]
            ks = [pp.tile([128, T], BF16, name=f"ks{h}") for h in range(HPC)]
            v1 = pp.tile([128, 32 * 65], BF16)  # [t-part, (jt, dv|1)] head 0
            v2 = pp.tile([128, 32 * 65], BF16)  # head 1
            attnT = pp.tile([128, T], BF16)  # [dv-part, t] normalized
            wo_g = pp.tile([128, DIM], BF16)
            ident = pp.tile([128, 128], BF16)
            identf = pp.tile([128, 128], F32)
            ones1 = pp.tile([1, 64], F32R)

            make_identity(nc, ident[:])
            make_identity(nc, identf[:])
            ones_f = pp.tile([128, 64], F32)
            nc.vector.memset(ones_f[:], 1.0)
            nc.vector.tensor_copy(ones1[:], ones_f[0:1, :])
            ones32 = pp.tile([128, 32], BF16)
            nc.vector.tensor_copy(ones32[:], ones_f[:, 0:32])
            # ones column at slot 64 of each 65-wide block of v1/v2; V
            # evacuations only write cols 0..63 of each block.
            for vv in (v1, v2):
                nc.vector.tensor_copy(
                    vv[:].rearrange("p (j c) -> p j c", c=65)[:, :, 64:65],
                    ones32[:].rearrange("p (j c) -> p j c", c=1),
                )
            # preload the exp activation table while DMAs run
            junk = pp.tile([1, 32], F32)
            nc.vector.memset(junk[:], 0.0)
            junk2 = pp.tile([1, 32], F32)
            nc.scalar.activation(junk2[:], junk[:], Exp)

            nc.sync.dma_start(wo_g[:], woT_d[:])

            blocks = [(b, ib) for b in range(B) for ib in range(2)]
            with (
                tc.tile_pool(name="esb", bufs=1) as ep,
                tc.tile_pool(name="s_ps", bufs=1, space="PSUM") as sps,
            ):
                ctx = {}  # ['pvps'/'sp'/'osp'] set when those pools open
                unorm, rcp, csd = {}, {}, {}
                flushbox = [False]
                pend = []  # closures: lagged PV groups + block evacuations

                def make_block(b, ib):
                    key = f"{b}_{ib}"
                    i0 = b * 2048 + ib * 1024
                    pv = []
                    e_pend = []

                    def emit_sact(jt):
                        j0 = b * 2048 + jt * 128
                        s_h = [
                            sps.tile([128, 1024], F32, tag=f"s{h}",
                                     name=f"s{key}_{jt}_{h}")
                            for h in range(2)
                        ]
                        e_h = [
                            ep.tile([128, 1024], BF16, tag=f"e{h}", bufs=12,
                                    name=f"e{key}_{jt}_{h}")
                            for h in range(2)
                        ]
                        # h-major: h0's S+exp only gate on ACT_h0(jt-1); the
                        # two heads' ACTs ping-pong and ScalarE stays busy
                        for h in range(2):
                            for ih in range(2):
                                nc.tensor.matmul(
                                    s_h[h][:, ih * 512 : (ih + 1) * 512],
                                    ks[h][:, j0 : j0 + 128],
                                    qs[h][:, i0 + ih * 512 : i0 + (ih + 1) * 512],
                                    start=True,
                                    stop=True,
                                )
                            nc.scalar.activation(e_h[h][:], s_h[h][:], Exp, scale=SCALE)
                        e_pend.append(e_h)
                        pend.append(lambda jt=jt: emit_pv(jt))

                    def emit_pv(jt):
                        if not pv:
                            pv.extend(
                                ctx["pvps"].tile([65, 1024], F32, tag=f"pv{h}",
                                                 name=f"pv{key}_{h}")
                                for h in range(2)
                            )
                        eh = e_pend.pop(0)
                        jv = (b * 16 + jt) * 65
                        for h, vv in enumerate((v1, v2)):
                            for ih in range(2):
                                nc.tensor.matmul(
                                    pv[h][:, ih * 512 : (ih + 1) * 512],
                                    vv[:, jv : jv + 65],
                                    eh[h][:, ih * 512 : (ih + 1) * 512],
                                    start=(jt == 0),
                                    stop=(jt == 15),
                                )

                    def emit_evac():
                        # denominator rows + unnormalized attn-out to SBUF;
                        # frees the pv psum banks (norm continues next block)
                        sp = ctx["sp"]
                        csd[key] = []
                        for h in range(2):
                            cs = sp.tile([1, 1024], F32, tag=f"cs{h}", name=f"cs{key}_{h}")
                            nc.vector.tensor_copy(cs[:], pv[h][64:65, :])
                            csd[key].append(cs)
                        unorm[key] = [
                            sp.tile([64, 1024], F32, tag=f"un{h}", name=f"un{key}_{h}")
                            for h in range(2)
                        ]
                        for h in range(2):
                            if flushbox[0] and h == 1:  # ScalarE idle in tail
                                nc.scalar.activation(unorm[key][h][:], pv[h][0:64, :], Copy)
                            else:
                                nc.vector.tensor_copy(unorm[key][h][:], pv[h][0:64, :])

                    return emit_sact, emit_evac

                def emit_boundary(pb, pib, step):
                    """Norm + out-projection for block (pb, pib), interleaved
                    into the next block's jt loop (or flushed at the end).
                    step 0: denominator transposes + reciprocal; 1: broadcast
                    + normalize; 2..5: two po pairs each."""
                    i0 = pb * 2048 + pib * 1024
                    key = f"{pb}_{pib}"
                    pvps, sp, osp = ctx["pvps"], ctx["sp"], ctx["osp"]
                    if step == 0:
                        # colsum rows -> partitions, reciprocal on 128 lanes,
                        # transpose back, DMA row-gather to [1, 1024]
                        pt = pvps.tile([128, 16], F32, tag="pv0", name=f"pt{key}")
                        for h in range(2):
                            for blk in range(8):
                                c = h * 8 + blk
                                nc.tensor.transpose(
                                    pt[:, c : c + 1],
                                    csd[key][h][0:1, blk * 128 : (blk + 1) * 128],
                                    identf[0:1, 0:1],
                                )
                        rT = sp.tile([128, 16], F32, tag="rT", name=f"rT{key}")
                        nc.vector.reciprocal(rT[:], pt[:])
                        pr = pvps.tile([16, 128], F32, tag="pv1", name=f"pr{key}")
                        nc.tensor.transpose(pr[:], rT[:], identf[:])
                        prs = sp.tile([16, 128], F32R, tag="prs", name=f"prs{key}")
                        nc.vector.tensor_copy(prs[:], pr[:])
                        r2 = [
                            sp.tile([1, 1024], F32R, tag=f"r{h}", name=f"r{key}_{h}")
                            for h in range(2)
                        ]
                        for h in range(2):
                            nc.sync.dma_start(r2[h][0:1, :], prs[h * 8 : (h + 1) * 8, :])
                        rcp[key] = r2
                    elif step == 1:
                        for h in range(2):
                            rbc = pvps.tile(
                                [64, 1024], F32, tag=f"pv{h}", name=f"rbc{key}_{h}"
                            )
                            for ih in range(2):
                                nc.tensor.matmul(
                                    rbc[:, ih * 512 : (ih + 1) * 512],
                                    ones1[:],
                                    rcp[key][h][0:1, ih * 512 : (ih + 1) * 512],
                                    start=True,
                                    stop=True,
                                )
                            rbs = sp.tile([64, 1024], F32, tag=f"rbs{h}", name=f"rbs{key}_{h}")
                            nc.vector.tensor_copy(rbs[:], rbc[:])
                            nc.vector.tensor_tensor(
                                attnT[h * 64 : (h + 1) * 64, i0 : i0 + 1024],
                                unorm[key][h][:],
                                rbs[:],
                                mult,
                            )
                    else:
                        for k in range(2):
                            tg = (step - 2) * 2 + k
                            row = i0 + tg * 128
                            po = pvps.tile(
                                [128, 1024], F32, tag=f"pv{k}", name=f"po{key}_{tg}"
                            )
                            for oh in range(2):
                                nc.tensor.matmul(
                                    po[:, oh * 512 : (oh + 1) * 512],
                                    attnT[:, row : row + 128],
                                    wo_g[:, oh * 512 : (oh + 1) * 512],
                                    start=True,
                                    stop=True,
                                )
                            ob = osp.tile(
                                [128, 1024], F32, tag="ob", bufs=6, name=f"ob{key}_{tg}"
                            )
                            # in the tail (flush) ScalarE is idle: split evacs
                            if flushbox[0] and tg % 2 == 1:
                                nc.scalar.activation(ob[:], po[:], Copy)
                            else:
                                nc.vector.tensor_copy(ob[:], po[:])
                            dmae = nc.sync if tg % 2 == 0 else nc.gpsimd
                            dmae.dma_start(out_d[row : row + 128, :], ob[:])

                sact0, evac0 = make_block(*blocks[0])

                # ---- Phase 1: QKV (+ V^T transposes), overlapped with the
                # ---- first attention block's S/exp
                with (
                    tc.tile_pool(name="ph1", bufs=1) as p1,
                    tc.tile_pool(name="qkv_ps", bufs=2, space="PSUM") as qkps,
                ):
                    wqkv_g = p1.tile([128, 8 * 384], BF16)  # [k-part, (kt, o)]
                    nc.sync.dma_start(
                        wqkv_g[:].rearrange("p (kt o) -> p kt o", kt=8),
                        wqkvT_d[:].rearrange("(kt p) o -> p kt o", p=128),
                    )
                    xq = [p1.tile([128, T], BF16, name=f"xq{i}") for i in range(8)]
                    vT = p1.tile([128, T], BF16)
                    # x arrives in t-quarter chunks so quarter-0 compute can
                    # start after ~1/4 of the x traffic
                    dmae = [nc.sync, nc.gpsimd, nc.scalar]
                    n = 0
                    for q in range(4):
                        for th in range(2 if q == 0 else 1):
                            w = 512 if q == 0 else 1024
                            for kt in range(8):
                                c0 = q * 1024 + th * 512
                                dmae[n % 3].dma_start(
                                    xq[kt][:, c0 : c0 + w],
                                    xT_d[kt * 128 : (kt + 1) * 128, c0 : c0 + w],
                                )
                                n += 1

                    sub = mybir.AluOpType.subtract

                    def emit_chain(q, ot, th):
                        ps = qkps.tile([128, 512], F32, tag="qk")
                        for kt in range(8):
                            nc.tensor.matmul(
                                ps[:],
                                wqkv_g[:, kt * 384 + ot * 128 : kt * 384 + (ot + 1) * 128],
                                xq[kt][:, q * 1024 + th * 512 : q * 1024 + (th + 1) * 512],
                                start=(kt == 0),
                                stop=(kt == 7),
                            )
                        col = q * 1024 + th * 512
                        cs_ = slice(col, col + 512)
                        if ot == 2:
                            nc.vector.tensor_copy(vT[:, cs_], ps[:])
                        elif ot == 0:  # q: hi = bf16(q), lo = q - hi
                            for hh in range(2):
                                php = ps[hh * 64 : (hh + 1) * 64, :]
                                nc.scalar.activation(qs[hh][0:64, cs_], php, Copy)
                                nc.vector.tensor_tensor(
                                    qs[hh][64:128, cs_], php, qs[hh][0:64, cs_], sub
                                )
                        else:  # k: duplicated into both array halves
                            for hh in range(2):
                                php = ps[hh * 64 : (hh + 1) * 64, :]
                                nc.scalar.activation(ks[hh][0:64, cs_], php, Copy)
                                nc.vector.tensor_copy(ks[hh][64:128, cs_], ks[hh][0:64, cs_])

                    def emit_vt(jt):
                        ptv = qkps.tile([128, 128], BF16, tag="vt", bufs=2)
                        nc.tensor.transpose(
                            ptv[:], vT[:, jt * 128 : (jt + 1) * 128], ident[:]
                        )
                        nc.vector.tensor_copy(v1[:, jt * 65 : jt * 65 + 64], ptv[:, 0:64])
                        nc.vector.tensor_copy(v2[:, jt * 65 : jt * 65 + 64], ptv[:, 64:128])

                    # quarters 0-1 (batch 0) straight through
                    for q in (0, 1):
                        for ot in (2, 1, 0):
                            for th in range(2):
                                emit_chain(q, ot, th)
                        for tj in range(8):
                            emit_vt(q * 8 + tj)
                    # block (0,0) S/exp interleaved with quarters 2-3; its PV
                    # is deferred (deep e-ring), so no pv psum needed yet
                    pieces = []
                    for q in (2, 3):
                        pieces += [
                            lambda q=q: emit_chain(q, 2, 0),
                            lambda q=q: emit_chain(q, 2, 1),
                            lambda q=q: emit_vt(q * 8 + 0),
                            lambda q=q: emit_vt(q * 8 + 1),
                            lambda q=q: emit_chain(q, 1, 0),
                            lambda q=q: emit_vt(q * 8 + 2),
                            lambda q=q: emit_vt(q * 8 + 3),
                            lambda q=q: emit_chain(q, 1, 1),
                            lambda q=q: emit_vt(q * 8 + 4),
                            lambda q=q: emit_vt(q * 8 + 5),
                            lambda q=q: emit_chain(q, 0, 0),
                            lambda q=q: emit_vt(q * 8 + 6),
                            lambda q=q: emit_vt(q * 8 + 7),
                            lambda q=q: emit_chain(q, 0, 1),
                        ]
                    npc = len(pieces)
                    for jt in range(8):
                        sact0(jt)
                        for _ in range((npc * (jt + 1)) // 8 - (npc * jt) // 8):
                            pieces.pop(0)()

                # ---- Phase 2: the rest of attention
                with (
                    tc.tile_pool(name="small", bufs=1) as sp,
                    tc.tile_pool(name="osb", bufs=1) as osp,
                    tc.tile_pool(name="pv_ps", bufs=1, space="PSUM") as pvps,
                ):
                    ctx["pvps"], ctx["sp"], ctx["osp"] = pvps, sp, osp

                    # block (0,0) key-tiles 8-15, draining its deferred PVs
                    # down to the steady-state queue depth
                    for jt in range(8, 16):
                        sact0(jt)
                        while len(pend) > 9:
                            pend.pop(0)()
                    pend.append(evac0)
                    prev = blocks[0]

                    # per-jt drain depth targets: PE slack fits ~1.2 PV groups
                    # per jt; pv allocations land at jt9, after the previous
                    # block's norm/po tiles in the psum rings.  The last block
                    # drains aggressively so the tail chain starts early.
                    TARGET = [9, 8, 7, 6, 5, 6, 7, 8, 9, 9, 9, 9, 9, 9, 9, 9]
                    TARGET_LAST = [9, 8, 7, 6, 5, 6, 7, 8, 9, 9, 8, 6, 4, 2, 1, 0]
                    for b, ib in blocks[1:]:
                        sact, evac = make_block(b, ib)
                        for jt in range(16):
                            sact(jt)
                            if jt in (5, 6):
                                emit_boundary(prev[0], prev[1], jt - 5)
                            elif jt == 7:
                                emit_boundary(prev[0], prev[1], 2)
                                emit_boundary(prev[0], prev[1], 3)
                            elif jt == 8:
                                emit_boundary(prev[0], prev[1], 4)
                                emit_boundary(prev[0], prev[1], 5)
                            tgt = TARGET_LAST if (b, ib) == blocks[-1] else TARGET
                            while len(pend) > tgt[jt]:
                                pend.pop(0)()
                        pend.append(evac)
                        prev = (b, ib)

                    # drain everything and flush the last block's norm + po
                    flushbox[0] = True
                    for f in pend:
                        f()
                    pend.clear()
                    for step in range(6):
                        emit_boundary(prev[0], prev[1], step)

    nc.compile()
    return nc


_NC = None


def _get_nc():
    global _NC
    if _NC is None:
        _NC = build_nc()
    return _NC


def _gate(mask):
    """Exact jax fp32 gate: sigmoid(m) > 0.5 (matches reference rounding)."""
    mask = np.asarray(mask, dtype=np.float32)
    return (np.float32(1.0) / (np.float32(1.0) + np.exp(-mask))) > np.float32(0.5)


def make_in_maps(x, qkv_weight, qkv_weight_mask, out_weight, out_weight_mask):
    import ml_dtypes

    bf16 = ml_dtypes.bfloat16
    x = np.asarray(x, dtype=np.float32)
    wqkv = np.where(_gate(qkv_weight_mask), np.asarray(qkv_weight, np.float32), 0.0)
    wo = np.where(_gate(out_weight_mask), np.asarray(out_weight, np.float32), 0.0)

    xT = np.ascontiguousarray(x.reshape(T, DIM).T).astype(bf16)
    in_maps = []
    for c in range(NCORES):
        r0 = c * DV
        sl = slice(r0, r0 + DV)
        w_shard = np.concatenate(
            [wqkv[sl], wqkv[DIM + r0 : DIM + r0 + DV], wqkv[2 * DIM + r0 : 2 * DIM + r0 + DV]],
            axis=0,
        )  # [384, 1024] rows = (q | k | v) for this core's 2 heads
        in_maps.append(
            {
                "xT": xT,
                "wqkvT": np.ascontiguousarray(w_shard.T).astype(bf16),
                "woT": np.ascontiguousarray(wo[:, sl].T).astype(bf16),
            }
        )
    return in_maps


LAST_RESULTS = None  # BassKernelResults of the most recent run (for profiling)


def kernel(
    x,
    qkv_weight,
    qkv_weight_mask,
    out_weight,
    out_weight_mask,
    out_bias,
    out_bias_mask,
    _trace=False,
    _tmpdir=None,
):
    global LAST_RESULTS
    from concourse.bass_utils import run_bass_kernel_spmd

    nc = _get_nc()
    in_maps = make_in_maps(x, qkv_weight, qkv_weight_mask, out_weight, out_weight_mask)
    res = run_bass_kernel_spmd(
        nc, in_maps, list(range(NCORES)), trace=_trace, tmpdir=_tmpdir
    )
    LAST_RESULTS = res
    out = np.zeros((T, DIM), dtype=np.float32)
    for r in res.results:
        out += r["out"]
    out_bias = np.asarray(out_bias, dtype=np.float32)
    out += np.where(_gate(out_bias_mask), out_bias, np.float32(0.0))[None, :]
    return out.reshape(B, N, DIM)


# revision 44
# speedup vs baseline: 1.0366x; 1.0366x over previous
"""Trainium2 Bass kernel for nn_Attention_41704132444382.

Masked-linear QKV projection + 16-head attention + masked-linear output
projection, tensor-parallel over heads across 8 NeuronCores (2 heads/core).

Design (ScalarE exp streaming ~128us is the roofline; PE ~ matches it):
  - Host: gates both masked-linear weights (sigmoid(m)>0.5), transposes x,
    casts x / wqkv / wo to bf16 (wqkv/wo values are +-c, near-exact in bf16).
  - QKV: xq bf16 tiles; lhsT = gated wqkv bf16 (FWL weight loads); psum
    [128,512] chains; q is stored split-precision (hi/lo bf16 pair) so the
    S matmul recovers fp32-exact q in the otherwise-idle half of the PE
    array (k is duplicated); k/q evacuated by ScalarE, v by DVE.  V^T is
    PE-transposed (bf16) to v1/v2 [t, dv|1] tiles whose ones column makes
    the PV matmul emit the softmax denominator for free (M=65).
  - The first attention block's S/exp overlaps the second half of QKV
    (its PV is deferred through a deep e-ring; QKV runs on 2 ping-pong
    psum banks during the overlap so s+qkv+vt fit in 8 banks).
  - Attention per 1024-query block, h-offset pipeline: per key-tile jt,
    one 1024-wide exp ACT per head (scale=1/32) -> e_h bf16; PV lags via
    a pend-queue drained to per-jt depth targets so block-boundary work
    (norm broadcast + out-projection) flows through the pv psum rings
    without stalling ScalarE.
  - Softmax denominators: pv row 64 -> [1,1024], PE-transpose chunks to
    partitions, DVE reciprocal, PE-transpose back, DMA row-gather, then
    a K=1 ones-matmul broadcast and one normalize tensor_tensor per head.
  - Output projection: lhsT = attnT bf16 (FWL), po pairs [128,1024] in
    the pv psum rings, DVE evac (split with ScalarE in the tail), DMA out.
"""

import os
import sys

import numpy as np

sys.path.insert(0, "/opt/trn_rl_repo")

import concourse.bass as bass
import concourse.mybir as mybir
from concourse import bacc
from concourse.masks import make_identity
from concourse.tile import TileContext

DIM = 1024
HEADS = 16
B = 2
N = 2048
T = B * N  # 4096 flattened tokens
NCORES = 8
HPC = HEADS // NCORES  # 2 heads per core
DV = HPC * 64  # 128 head-dims per core
SCALE = DIM ** (-0.5)  # 1/32

F32 = mybir.dt.float32
F32R = mybir.dt.float32r
BF16 = mybir.dt.bfloat16

Copy = mybir.ActivationFunctionType.Copy
Exp = mybir.ActivationFunctionType.Exp
mult = mybir.AluOpType.mult


def build_nc():
    nc = bacc.Bacc("TRN2", target_bir_lowering=True)
    xT_d = nc.declare_dram_parameter("xT", [DIM, T], BF16, isOutput=False)
    wqkvT_d = nc.declare_dram_parameter("wqkvT", [DIM, 384], BF16, isOutput=False)
    woT_d = nc.declare_dram_parameter("woT", [DV, DIM], BF16, isOutput=False)
    out_d = nc.declare_dram_parameter("out", [T, DIM], F32, isOutput=True)

    with TileContext(nc) as tc:
        with tc.tile_pool(name="persist", bufs=1) as pp:
            qs = [pp.tile([128, T], BF16, name=f"qs{h}") for h in range(HPC)]
            ks = [pp.tile([128, T], BF16, name=f"ks{h}") for h in range(HPC)]
            v1 = pp.tile([128, 32 * 65], BF16)  # [t-part, (jt, dv|1)] head 0
            v2 = pp.tile([128, 32 * 65], BF16)  # head 1
            attnT = pp.tile([128, T], BF16)  # [dv-part, t] normalized
            wo_g = pp.tile([128, DIM], BF16)
            ident = pp.tile([128, 128], BF16)
            identf = pp.tile([128, 128], F32)
            ones1 = pp.tile([1, 64], F32R)

            make_identity(nc, ident[:])
            make_identity(nc, identf[:])
            ones_f = pp.tile([128, 64], F32)
            nc.vector.memset(ones_f[:], 1.0)
            nc.vector.tensor_copy(ones1[:], ones_f[0:1, :])
            ones32 = pp.tile([128, 32], BF16)
            nc.vector.tensor_copy(ones32[:], ones_f[:, 0:32])
            # ones column at slot 64 of each 65-wide block of v1/v2; V
            # evacuations only write cols 0..63 of each block.
            for vv in (v1, v2):
                nc.vector.tensor_copy(
                    vv[:].rearrange("p (j c) -> p j c", c=65)[:, :, 64:65],
                    ones32[:].rearrange("p (j c) -> p j c", c=1),
                )
            # preload the exp activation table while DMAs run
            junk = pp.tile([1, 32], F32)
            nc.vector.memset(junk[:], 0.0)
            junk2 = pp.tile([1, 32], F32)
            nc.scalar.activation(junk2[:], junk[:], Exp)

            nc.sync.dma_start(wo_g[:], woT_d[:])

            blocks = [(b, ib) for b in range(B) for ib in range(2)]
            with (
                tc.tile_pool(name="esb", bufs=1) as ep,
                tc.tile_pool(name="s_ps", bufs=1, space="PSUM") as sps,
            ):
                ctx = {}  # ['pvps'/'sp'/'osp'] set when those pools open
                unorm, rcp, csd = {}, {}, {}
                flushbox = [False]
                pend = []  # closures: lagged PV groups + block evacuations

                def make_block(b, ib):
                    key = f"{b}_{ib}"
                    i0 = b * 2048 + ib * 1024
                    pv = []
                    e_pend = []

                    def emit_sact(jt):
                        j0 = b * 2048 + jt * 128
                        s_h = [
                            sps.tile([128, 1024], F32, tag=f"s{h}",
                                     name=f"s{key}_{jt}_{h}")
                            for h in range(2)
                        ]
                        e_h = [
                            ep.tile([128, 1024], BF16, tag=f"e{h}", bufs=15,
                                    name=f"e{key}_{jt}_{h}")
                            for h in range(2)
                        ]
                        # h-major: h0's S+exp only gate on ACT_h0(jt-1); the
                        # two heads' ACTs ping-pong and ScalarE stays busy
                        for h in range(2):
                            for ih in range(2):
                                nc.tensor.matmul(
                                    s_h[h][:, ih * 512 : (ih + 1) * 512],
                                    ks[h][:, j0 : j0 + 128],
                                    qs[h][:, i0 + ih * 512 : i0 + (ih + 1) * 512],
                                    start=True,
                                    stop=True,
                                )
                            nc.scalar.activation(e_h[h][:], s_h[h][:], Exp, scale=SCALE)
                        e_pend.append(e_h)
                        pend.append(lambda jt=jt: emit_pv(jt))

                    def emit_pv(jt):
                        if not pv:
                            pv.extend(
                                ctx["pvps"].tile([65, 1024], F32, tag=f"pv{h}",
                                                 name=f"pv{key}_{h}")
                                for h in range(2)
                            )
                        eh = e_pend.pop(0)
                        jv = (b * 16 + jt) * 65
                        for h, vv in enumerate((v1, v2)):
                            for ih in range(2):
                                nc.tensor.matmul(
                                    pv[h][:, ih * 512 : (ih + 1) * 512],
                                    vv[:, jv : jv + 65],
                                    eh[h][:, ih * 512 : (ih + 1) * 512],
                                    start=(jt == 0),
                                    stop=(jt == 15),
                                )

                    def emit_evac():
                        # denominator rows + unnormalized attn-out to SBUF;
                        # frees the pv psum banks (norm continues next block)
                        sp = ctx["sp"]
                        csd[key] = []
                        for h in range(2):
                            cs = sp.tile([1, 1024], F32, tag=f"cs{h}", name=f"cs{key}_{h}")
                            nc.vector.tensor_copy(cs[:], pv[h][64:65, :])
                            csd[key].append(cs)
                        unorm[key] = [
                            sp.tile([64, 1024], F32, tag=f"un{h}", name=f"un{key}_{h}")
                            for h in range(2)
                        ]
                        for h in range(2):
                            if flushbox[0] and h == 1:  # ScalarE idle in tail
                                nc.scalar.activation(unorm[key][h][:], pv[h][0:64, :], Copy)
                            else:
                                nc.vector.tensor_copy(unorm[key][h][:], pv[h][0:64, :])

                    return emit_sact, emit_evac

                def emit_boundary(pb, pib, step):
                    """Norm + out-projection for block (pb, pib), interleaved
                    into the next block's jt loop (or flushed at the end).
                    step 0: denominator transposes + reciprocal; 1: broadcast
                    + normalize; 2..5: two po pairs each."""
                    i0 = pb * 2048 + pib * 1024
                    key = f"{pb}_{pib}"
                    pvps, sp, osp = ctx["pvps"], ctx["sp"], ctx["osp"]
                    if step == 0:
                        # colsum rows -> partitions, reciprocal on 128 lanes,
                        # transpose back, DMA row-gather to [1, 1024]
                        pt = pvps.tile([128, 16], F32, tag="pv0", name=f"pt{key}")
                        for h in range(2):
                            for blk in range(8):
                                c = h * 8 + blk
                                nc.tensor.transpose(
                                    pt[:, c : c + 1],
                                    csd[key][h][0:1, blk * 128 : (blk + 1) * 128],
                                    identf[0:1, 0:1],
                                )
                        rT = sp.tile([128, 16], F32, tag="rT", name=f"rT{key}")
                        nc.vector.reciprocal(rT[:], pt[:])
                        pr = pvps.tile([16, 128], F32, tag="pv1", name=f"pr{key}")
                        nc.tensor.transpose(pr[:], rT[:], identf[:])
                        prs = sp.tile([16, 128], F32R, tag="prs", name=f"prs{key}")
                        nc.vector.tensor_copy(prs[:], pr[:])
                        r2 = [
                            sp.tile([1, 1024], F32R, tag=f"r{h}", name=f"r{key}_{h}")
                            for h in range(2)
                        ]
                        for h in range(2):
                            nc.sync.dma_start(r2[h][0:1, :], prs[h * 8 : (h + 1) * 8, :])
                        rcp[key] = r2
                    elif step == 1:
                        for h in range(2):
                            rbc = pvps.tile(
                                [64, 1024], F32, tag=f"pv{h}", name=f"rbc{key}_{h}"
                            )
                            for ih in range(2):
                                nc.tensor.matmul(
                                    rbc[:, ih * 512 : (ih + 1) * 512],
                                    ones1[:],
                                    rcp[key][h][0:1, ih * 512 : (ih + 1) * 512],
                                    start=True,
                                    stop=True,
                                )
                            rbs = sp.tile([64, 1024], F32, tag=f"rbs{h}", name=f"rbs{key}_{h}")
                            nc.vector.tensor_copy(rbs[:], rbc[:])
                            nc.vector.tensor_tensor(
                                attnT[h * 64 : (h + 1) * 64, i0 : i0 + 1024],
                                unorm[key][h][:],
                                rbs[:],
                                mult,
                            )
                    else:
                        for k in range(2):
                            tg = (step - 2) * 2 + k
                            row = i0 + tg * 128
                            po = pvps.tile(
                                [128, 1024], F32, tag=f"pv{k}", name=f"po{key}_{tg}"
                            )
                            for oh in range(2):
                                nc.tensor.matmul(
                                    po[:, oh * 512 : (oh + 1) * 512],
                                    attnT[:, row : row + 128],
                                    wo_g[:, oh * 512 : (oh + 1) * 512],
                                    start=True,
                                    stop=True,
                                )
                            ob = osp.tile(
                                [128, 1024], F32, tag="ob", bufs=6, name=f"ob{key}_{tg}"
                            )
                            # in the tail (flush) ScalarE is idle: split evacs
                            if flushbox[0] and tg % 2 == 1:
                                nc.scalar.activation(ob[:], po[:], Copy)
                            else:
                                nc.vector.tensor_copy(ob[:], po[:])
                            dmae = nc.sync if tg % 2 == 0 else nc.gpsimd
                            dmae.dma_start(out_d[row : row + 128, :], ob[:])

                sact0, evac0 = make_block(*blocks[0])

                # ---- Phase 1: QKV (+ V^T transposes), overlapped with the
                # ---- first attention block's S/exp
                with (
                    tc.tile_pool(name="ph1", bufs=1) as p1,
                    tc.tile_pool(name="qkv_ps", bufs=2, space="PSUM") as qkps,
                ):
                    wqkv_g = p1.tile([128, 8 * 384], BF16)  # [k-part, (kt, o)]
                    nc.sync.dma_start(
                        wqkv_g[:].rearrange("p (kt o) -> p kt o", kt=8),
                        wqkvT_d[:].rearrange("(kt p) o -> p kt o", p=128),
                    )
                    xq = [p1.tile([128, T], BF16, name=f"xq{i}") for i in range(8)]
                    vT = p1.tile([128, T], BF16)
                    # x arrives in t-quarter chunks so quarter-0 compute can
                    # start after ~1/4 of the x traffic
                    dmae = [nc.sync, nc.gpsimd, nc.scalar]
                    n = 0
                    for q in range(4):
                        for th in range(2 if q == 0 else 1):
                            w = 512 if q == 0 else 1024
                            for kt in range(8):
                                c0 = q * 1024 + th * 512
                                dmae[n % 3].dma_start(
                                    xq[kt][:, c0 : c0 + w],
                                    xT_d[kt * 128 : (kt + 1) * 128, c0 : c0 + w],
                                )
                                n += 1

                    sub = mybir.AluOpType.subtract

                    def emit_chain(q, ot, th):
                        ps = qkps.tile([128, 512], F32, tag="qk")
                        for kt in range(8):
                            nc.tensor.matmul(
                                ps[:],
                                wqkv_g[:, kt * 384 + ot * 128 : kt * 384 + (ot + 1) * 128],
                                xq[kt][:, q * 1024 + th * 512 : q * 1024 + (th + 1) * 512],
                                start=(kt == 0),
                                stop=(kt == 7),
                            )
                        col = q * 1024 + th * 512
                        cs_ = slice(col, col + 512)
                        # during quarters 2-3 ScalarE is running block-0 exps:
                        # keep those quarters' evacuations off it
                        sc = nc.scalar.activation if q < 2 else (
                            lambda o, i, _f: nc.vector.tensor_copy(o, i)
                        )
                        if ot == 2:
                            nc.vector.tensor_copy(vT[:, cs_], ps[:])
                        elif ot == 0:  # q: hi = bf16(q), lo = q - hi
                            for hh in range(2):
                                php = ps[hh * 64 : (hh + 1) * 64, :]
                                sc(qs[hh][0:64, cs_], php, Copy)
                                nc.vector.tensor_tensor(
                                    qs[hh][64:128, cs_], php, qs[hh][0:64, cs_], sub
                                )
                        else:  # k: duplicated into both array halves
                            for hh in range(2):
                                php = ps[hh * 64 : (hh + 1) * 64, :]
                                sc(ks[hh][0:64, cs_], php, Copy)
                                nc.vector.tensor_copy(ks[hh][64:128, cs_], ks[hh][0:64, cs_])

                    def emit_vt(jt):
                        ptv = qkps.tile([128, 128], BF16, tag="vt", bufs=2)
                        nc.tensor.transpose(
                            ptv[:], vT[:, jt * 128 : (jt + 1) * 128], ident[:]
                        )
                        nc.vector.tensor_copy(v1[:, jt * 65 : jt * 65 + 64], ptv[:, 0:64])
                        nc.vector.tensor_copy(v2[:, jt * 65 : jt * 65 + 64], ptv[:, 64:128])

                    # quarters 0-1 (batch 0) straight through
                    for q in (0, 1):
                        for ot in (2, 1, 0):
                            for th in range(2):
                                emit_chain(q, ot, th)
                        for tj in range(8):
                            emit_vt(q * 8 + tj)
                    # block (0,0) S/exp interleaved with quarters 2-3; its PV
                    # is deferred (deep e-ring), so no pv psum needed yet
                    pieces = []
                    for q in (2, 3):
                        pieces += [
                            lambda q=q: emit_chain(q, 2, 0),
                            lambda q=q: emit_chain(q, 2, 1),
                            lambda q=q: emit_vt(q * 8 + 0),
                            lambda q=q: emit_vt(q * 8 + 1),
                            lambda q=q: emit_chain(q, 1, 0),
                            lambda q=q: emit_vt(q * 8 + 2),
                            lambda q=q: emit_vt(q * 8 + 3),
                            lambda q=q: emit_chain(q, 1, 1),
                            lambda q=q: emit_vt(q * 8 + 4),
                            lambda q=q: emit_vt(q * 8 + 5),
                            lambda q=q: emit_chain(q, 0, 0),
                            lambda q=q: emit_vt(q * 8 + 6),
                            lambda q=q: emit_vt(q * 8 + 7),
                            lambda q=q: emit_chain(q, 0, 1),
                        ]
                    npc = len(pieces)
                    for jt in range(8):
                        sact0(jt)
                        for _ in range((npc * (jt + 1)) // 8 - (npc * jt) // 8):
                            pieces.pop(0)()

                # ---- Phase 2: the rest of attention
                with (
                    tc.tile_pool(name="small", bufs=1) as sp,
                    tc.tile_pool(name="osb", bufs=1) as osp,
                    tc.tile_pool(name="pv_ps", bufs=1, space="PSUM") as pvps,
                ):
                    ctx["pvps"], ctx["sp"], ctx["osp"] = pvps, sp, osp

                    # block (0,0) key-tiles 8-15, draining its deferred PVs
                    # down to the steady-state queue depth
                    for jt in range(8, 16):
                        sact0(jt)
                        while len(pend) > 9:
                            pend.pop(0)()
                    pend.append(evac0)
                    prev = blocks[0]

                    # per-jt drain depth targets: PE slack fits ~1.2 PV groups
                    # per jt; pv allocations land at jt9, after the previous
                    # block's norm/po tiles in the psum rings.  The last block
                    # drains aggressively so the tail chain starts early.
                    TARGET = [9, 8, 7, 6, 5, 6, 7, 8, 9, 9, 9, 9, 9, 9, 9, 9]
                    TARGET_LAST = [9, 8, 7, 6, 5, 6, 7, 8, 9, 9, 8, 6, 4, 2, 1, 0]
                    for b, ib in blocks[1:]:
                        sact, evac = make_block(b, ib)
                        for jt in range(16):
                            sact(jt)
                            if jt in (5, 6):
                                emit_boundary(prev[0], prev[1], jt - 5)
                            elif jt == 7:
                                emit_boundary(prev[0], prev[1], 2)
                                emit_boundary(prev[0], prev[1], 3)
                            elif jt == 8:
                                emit_boundary(prev[0], prev[1], 4)
                                emit_boundary(prev[0], prev[1], 5)
                            tgt = TARGET_LAST if (b, ib) == blocks[-1] else TARGET
                            while len(pend) > tgt[jt]:
                                pend.pop(0)()
                        pend.append(evac)
                        prev = (b, ib)

                    # drain everything and flush the last block's norm + po
                    flushbox[0] = True
                    for f in pend:
                        f()
                    pend.clear()
                    for step in range(6):
                        emit_boundary(prev[0], prev[1], step)

    nc.compile()
    return nc


_NC = None


def _get_nc():
    global _NC
    if _NC is None:
        _NC = build_nc()
    return _NC


def _gate(mask):
    """Exact jax fp32 gate: sigmoid(m) > 0.5 (matches reference rounding)."""
    mask = np.asarray(mask, dtype=np.float32)
    return (np.float32(1.0) / (np.float32(1.0) + np.exp(-mask))) > np.float32(0.5)


def make_in_maps(x, qkv_weight, qkv_weight_mask, out_weight, out_weight_mask):
    import ml_dtypes

    bf16 = ml_dtypes.bfloat16
    x = np.asarray(x, dtype=np.float32)
    wqkv = np.where(_gate(qkv_weight_mask), np.asarray(qkv_weight, np.float32), 0.0)
    wo = np.where(_gate(out_weight_mask), np.asarray(out_weight, np.float32), 0.0)

    xT = np.ascontiguousarray(x.reshape(T, DIM).T).astype(bf16)
    in_maps = []
    for c in range(NCORES):
        r0 = c * DV
        sl = slice(r0, r0 + DV)
        w_shard = np.concatenate(
            [wqkv[sl], wqkv[DIM + r0 : DIM + r0 + DV], wqkv[2 * DIM + r0 : 2 * DIM + r0 + DV]],
            axis=0,
        )  # [384, 1024] rows = (q | k | v) for this core's 2 heads
        in_maps.append(
            {
                "xT": xT,
                "wqkvT": np.ascontiguousarray(w_shard.T).astype(bf16),
                "woT": np.ascontiguousarray(wo[:, sl].T).astype(bf16),
            }
        )
    return in_maps


LAST_RESULTS = None  # BassKernelResults of the most recent run (for profiling)


def kernel(
    x,
    qkv_weight,
    qkv_weight_mask,
    out_weight,
    out_weight_mask,
    out_bias,
    out_bias_mask,
    _trace=False,
    _tmpdir=None,
):
    global LAST_RESULTS
    from concourse.bass_utils import run_bass_kernel_spmd

    nc = _get_nc()
    in_maps = make_in_maps(x, qkv_weight, qkv_weight_mask, out_weight, out_weight_mask)
    res = run_bass_kernel_spmd(
        nc, in_maps, list(range(NCORES)), trace=_trace, tmpdir=_tmpdir
    )
    LAST_RESULTS = res
    out = np.zeros((T, DIM), dtype=np.float32)
    for r in res.results:
        out += r["out"]
    out_bias = np.asarray(out_bias, dtype=np.float32)
    out += np.where(_gate(out_bias_mask), out_bias, np.float32(0.0))[None, :]
    return out.reshape(B, N, DIM)


# revision 45
# speedup vs baseline: 1.1558x; 1.1150x over previous
"""Trainium2 Bass kernel for nn_Attention_41704132444382.

Masked-linear QKV projection + 16-head attention + masked-linear output
projection, tensor-parallel over heads across 8 NeuronCores (2 heads/core).

Design (ScalarE exp streaming ~135us is the roofline; PE ~ matches it):
  - Host: gates both masked-linear weights (sigmoid(m)>0.5), transposes x,
    casts x / wqkv / wo to bf16 (wqkv/wo values are +-c, near-exact in bf16).
  - QKV: xq bf16 tiles; lhsT = gated wqkv bf16 (FWL weight loads); psum
    [128,512] chains; q is stored split-precision (hi/lo bf16 pair) so the
    S matmul recovers fp32-exact q in the otherwise-idle half of the PE
    array (k is duplicated); k/q evacuated by ScalarE, v by DVE.  V^T is
    PE-transposed (bf16) to v1/v2 [t, dv|1] tiles whose ones column makes
    the PV matmul emit the softmax denominator for free (M=65).
  - Attention per 1024-query block, h-offset pipeline: per key-tile jt,
    S as K=128 all-bf16 matmuls, one 1024-wide exp ACT per head
    (scale=1/32) -> e_h bf16; PV lags via a pend-queue drained to per-jt
    depth targets so block-boundary work (norm broadcast + out-projection)
    flows through the pv psum rings without stalling ScalarE.
  - Softmax denominators: pv row 64 -> [1,1024], PE-transpose chunks to
    partitions, DVE reciprocal, PE-transpose back, DMA row-gather, then
    a K=1 ones-matmul broadcast and one normalize tensor_tensor per head.
  - Output projection: lhsT = attnT bf16 (FWL), po pairs [128,1024] in
    the pv psum rings, evacs split DVE/ScalarE, DMA out from SBUF.
"""

import os
import sys

import numpy as np

sys.path.insert(0, "/opt/trn_rl_repo")

import concourse.bass as bass
import concourse.mybir as mybir
from concourse import bacc
from concourse.masks import make_identity
from concourse.tile import TileContext

DIM = 1024
HEADS = 16
B = 2
N = 2048
T = B * N  # 4096 flattened tokens
NCORES = 8
HPC = HEADS // NCORES  # 2 heads per core
DV = HPC * 64  # 128 head-dims per core
SCALE = DIM ** (-0.5)  # 1/32

F32 = mybir.dt.float32
F32R = mybir.dt.float32r
BF16 = mybir.dt.bfloat16

Copy = mybir.ActivationFunctionType.Copy
Exp = mybir.ActivationFunctionType.Exp
mult = mybir.AluOpType.mult


def build_nc():
    nc = bacc.Bacc("TRN2", target_bir_lowering=True)
    xT_d = nc.declare_dram_parameter("xT", [DIM, T], BF16, isOutput=False)
    wqkvT_d = nc.declare_dram_parameter("wqkvT", [DIM, 384], BF16, isOutput=False)
    woT_d = nc.declare_dram_parameter("woT", [DV, DIM], BF16, isOutput=False)
    out_d = nc.declare_dram_parameter("out", [T, DIM], F32, isOutput=True)

    with TileContext(nc) as tc:
        with tc.tile_pool(name="persist", bufs=1) as pp:
            # S runs as K=128 all-bf16 matmuls (FWL weight loads keep PE array
            # duty high -> HAM stays at 2.4 GHz) with split-precision q in the
            # otherwise-idle half of the array: qs rows 0-63 = bf16(q), rows
            # 64-127 = bf16(q - bf16(q)); ks duplicates k in both halves, so
            # k.T q accumulates the hi and lo products -> q is fp32-exact.
            qs = [pp.tile([128, T], BF16, name=f"qs{h}") for h in range(HPC)]
            ks = [pp.tile([128, T], BF16, name=f"ks{h}") for h in range(HPC)]
            v1 = pp.tile([128, 32 * 65], BF16)  # [t-part, (jt, dv|1)] head 0
            v2 = pp.tile([128, 32 * 65], BF16)  # head 1
            attnT = pp.tile([128, T], BF16)  # [dv-part, t] normalized
            wo_g = pp.tile([128, DIM], BF16)
            ident = pp.tile([128, 128], BF16)
            identf = pp.tile([128, 128], F32)
            ones1 = pp.tile([1, 64], F32R)

            make_identity(nc, ident[:])
            make_identity(nc, identf[:])
            ones_f = pp.tile([128, 64], F32)
            nc.vector.memset(ones_f[:], 1.0)
            nc.vector.tensor_copy(ones1[:], ones_f[0:1, :])
            ones32 = pp.tile([128, 32], BF16)
            nc.vector.tensor_copy(ones32[:], ones_f[:, 0:32])
            # ones column at slot 64 of each 65-wide block of v1/v2; V
            # evacuations only write cols 0..63 of each block.
            for vv in (v1, v2):
                nc.vector.tensor_copy(
                    vv[:].rearrange("p (j c) -> p j c", c=65)[:, :, 64:65],
                    ones32[:].rearrange("p (j c) -> p j c", c=1),
                )
            # preload the exp activation table while DMAs run
            junk = pp.tile([1, 32], F32)
            nc.vector.memset(junk[:], 0.0)
            junk2 = pp.tile([1, 32], F32)
            nc.scalar.activation(junk2[:], junk[:], Exp)

            # ---------- Phase 1: QKV projection (+ V^T transpose) ----------
            with (
                tc.tile_pool(name="ph1", bufs=1) as p1,
                tc.tile_pool(name="qkv_ps", bufs=4, space="PSUM") as qkps,
            ):
                wqkv_g = p1.tile([128, 8 * 384], BF16)  # [k-part, (kt, o)]
                nc.gpsimd.dma_start(
                    wqkv_g[:].rearrange("p (kt o) -> p kt o", kt=8),
                    wqkvT_d[:].rearrange("(kt p) o -> p kt o", p=128),
                )
                nc.gpsimd.dma_start(wo_g[:], woT_d[:])
                xq = [p1.tile([128, T], BF16, name=f"xq{i}") for i in range(8)]
                vT = p1.tile([128, T], BF16)
                # quarter 0 gets a dedicated queue + fine chunks so its
                # kt-chains start as soon as each slice lands
                for th in range(2):
                    for kt in range(8):
                        c0 = th * 512
                        nc.sync.dma_start(
                            xq[kt][:, c0 : c0 + 512],
                            xT_d[kt * 128 : (kt + 1) * 128, c0 : c0 + 512],
                        )
                dmae = [nc.gpsimd, nc.scalar]
                n = 0
                for q in range(1, 4):
                    for kt in range(8):
                        c0 = q * 1024
                        dmae[n % 2].dma_start(
                            xq[kt][:, c0 : c0 + 1024],
                            xT_d[kt * 128 : (kt + 1) * 128, c0 : c0 + 1024],
                        )
                        n += 1

                sub = mybir.AluOpType.subtract
                for q in range(4):
                    # v first so transposes can interleave with q/k matmuls
                    for ot in (2, 1, 0):
                        for th in range(2):
                            ps = qkps.tile([128, 512], F32, tag="qk")
                            for kt in range(8):
                                nc.tensor.matmul(
                                    ps[:],
                                    wqkv_g[:, kt * 384 + ot * 128 : kt * 384 + (ot + 1) * 128],
                                    xq[kt][:, q * 1024 + th * 512 : q * 1024 + (th + 1) * 512],
                                    start=(kt == 0),
                                    stop=(kt == 7),
                                )
                            col = q * 1024 + th * 512
                            cs_ = slice(col, col + 512)
                            if ot == 2:
                                nc.vector.tensor_copy(vT[:, cs_], ps[:])
                            elif ot == 0:  # q: hi = bf16(q), lo = q - hi
                                for hh in range(2):
                                    php = ps[hh * 64 : (hh + 1) * 64, :]
                                    nc.scalar.activation(qs[hh][0:64, cs_], php, Copy)
                                    nc.vector.tensor_tensor(
                                        qs[hh][64:128, cs_], php, qs[hh][0:64, cs_], sub
                                    )
                            else:  # k: duplicated into both array halves
                                for hh in range(2):
                                    php = ps[hh * 64 : (hh + 1) * 64, :]
                                    nc.scalar.activation(ks[hh][0:64, cs_], php, Copy)
                                    nc.vector.tensor_copy(ks[hh][64:128, cs_], ks[hh][0:64, cs_])
                    for tj in range(8):  # V^T -> v1/v2 for this quarter
                        jt = q * 8 + tj
                        ptv = qkps.tile([128, 128], BF16, tag="vt", bufs=2)
                        nc.tensor.transpose(
                            ptv[:], vT[:, jt * 128 : (jt + 1) * 128], ident[:]
                        )
                        nc.vector.tensor_copy(v1[:, jt * 65 : jt * 65 + 64], ptv[:, 0:64])
                        nc.vector.tensor_copy(v2[:, jt * 65 : jt * 65 + 64], ptv[:, 64:128])

            # ---------- Phase 2: attention ----------
            with (
                tc.tile_pool(name="esb", bufs=1) as ep,
                tc.tile_pool(name="small", bufs=1) as sp,
                tc.tile_pool(name="osb", bufs=1) as osp,
                tc.tile_pool(name="s_ps", bufs=1, space="PSUM") as sps,
                tc.tile_pool(name="pv_ps", bufs=1, space="PSUM") as pvps,
            ):
                blocks = [(b, ib) for b in range(B) for ib in range(2)]

                def emit_boundary(pb, pib, step):
                    """Norm + out-projection for block (pb, pib), interleaved
                    into the next block's jt loop (or flushed at the end).
                    step 0: denominator transposes + reciprocal; 1: broadcast
                    + normalize; 2..5: two po pairs each."""
                    i0 = pb * 2048 + pib * 1024
                    key = f"{pb}_{pib}"
                    if step == 0:
                        # colsum rows -> partitions, reciprocal on 128 lanes,
                        # transpose back, DMA row-gather to [1, 1024]
                        pt = pvps.tile([128, 16], F32, tag="pv0", name=f"pt{key}")
                        for h in range(2):
                            for blk in range(8):
                                c = h * 8 + blk
                                nc.tensor.transpose(
                                    pt[:, c : c + 1],
                                    csd[key][h][0:1, blk * 128 : (blk + 1) * 128],
                                    identf[0:1, 0:1],
                                )
                        rT = sp.tile([128, 16], F32, tag="rT", name=f"rT{key}")
                        nc.vector.reciprocal(rT[:], pt[:])
                        pr = pvps.tile([16, 128], F32, tag="pv1", name=f"pr{key}")
                        nc.tensor.transpose(pr[:], rT[:], identf[:])
                        prs = sp.tile([16, 128], F32R, tag="prs", name=f"prs{key}")
                        nc.vector.tensor_copy(prs[:], pr[:])
                        r2 = [
                            sp.tile([1, 1024], F32R, tag=f"r{h}", name=f"r{key}_{h}")
                            for h in range(2)
                        ]
                        for h in range(2):
                            nc.sync.dma_start(r2[h][0:1, :], prs[h * 8 : (h + 1) * 8, :])
                        rcp[key] = r2
                    elif step == 1:
                        for h in range(2):
                            rbc = pvps.tile(
                                [64, 1024], F32, tag=f"pv{h}", name=f"rbc{key}_{h}"
                            )
                            for ih in range(2):
                                nc.tensor.matmul(
                                    rbc[:, ih * 512 : (ih + 1) * 512],
                                    ones1[:],
                                    rcp[key][h][0:1, ih * 512 : (ih + 1) * 512],
                                    start=True,
                                    stop=True,
                                )
                            rbs = sp.tile([64, 1024], F32, tag=f"rbs{h}", name=f"rbs{key}_{h}")
                            nc.vector.tensor_copy(rbs[:], rbc[:])
                            nc.vector.tensor_tensor(
                                attnT[h * 64 : (h + 1) * 64, i0 : i0 + 1024],
                                unorm[key][h][:],
                                rbs[:],
                                mult,
                            )
                    else:
                        for k in range(2):
                            tg = (step - 2) * 2 + k
                            row = i0 + tg * 128
                            po = pvps.tile(
                                [128, 1024], F32, tag=f"pv{k}", name=f"po{key}_{tg}"
                            )
                            for oh in range(2):
                                nc.tensor.matmul(
                                    po[:, oh * 512 : (oh + 1) * 512],
                                    attnT[:, row : row + 128],
                                    wo_g[:, oh * 512 : (oh + 1) * 512],
                                    start=True,
                                    stop=True,
                                )
                            ob = osp.tile(
                                [128, 1024], F32, tag="ob", bufs=6, name=f"ob{key}_{tg}"
                            )
                            # ScalarE stalls at boundaries anyway; splitting
                            # the po evacuations halves the pv-ring drain
                            if tg % 2 == 1:
                                nc.scalar.activation(ob[:], po[:], Copy)
                            else:
                                nc.vector.tensor_copy(ob[:], po[:])
                            dmae = nc.sync if tg % 2 == 0 else nc.gpsimd
                            dmae.dma_start(out_d[row : row + 128, :], ob[:])

                unorm = {}
                rcp = {}
                csd = {}
                flush = False
                pend = []  # closures: lagged PV groups + block-end evacuations
                # drain to a per-jt target queue depth: PE slack per jt fits
                # ~1.2 PV groups, so each block's PV tail spills into the next
                # block's early key-tiles (norm at jt5-6, po pairs at jt7-8);
                # the targets keep pv allocations at jt9, after the previous
                # block's norm/po tiles in the psum rings.  The last block
                # drains aggressively so the tail chain starts early.
                TARGET = [9, 8, 7, 6, 5, 6, 7, 8, 9, 9, 9, 9, 9, 9, 9, 9]
                TARGET_LAST = [9, 8, 7, 6, 5, 6, 7, 8, 9, 9, 8, 6, 4, 2, 1, 0]
                prev = None
                for b, ib in blocks:
                    key = f"{b}_{ib}"
                    i0 = b * 2048 + ib * 1024
                    # allocated lazily at the first emit_pv so the pv-ring
                    # order is: prev block's pv -> prev's rbc/po -> ours
                    pv = []
                    e_pend = []

                    def emit_pv(jt, key=key, b=b, pv=pv, e_pend=e_pend):
                        if not pv:
                            pv.extend(
                                pvps.tile([65, 1024], F32, tag=f"pv{h}", name=f"pv{key}_{h}")
                                for h in range(2)
                            )
                        eh = e_pend.pop(0)
                        jv = (b * 16 + jt) * 65
                        for h, vv in enumerate((v1, v2)):
                            for ih in range(2):
                                nc.tensor.matmul(
                                    pv[h][:, ih * 512 : (ih + 1) * 512],
                                    vv[:, jv : jv + 65],
                                    eh[h][:, ih * 512 : (ih + 1) * 512],
                                    start=(jt == 0),
                                    stop=(jt == 15),
                                )

                    def emit_evac(key=key, pv=pv):
                        # denominator rows + unnormalized attn-out to SBUF;
                        # frees the pv psum banks (norm continues next block)
                        csd[key] = []
                        for h in range(2):
                            cs = sp.tile([1, 1024], F32, tag=f"cs{h}", name=f"cs{key}_{h}")
                            nc.vector.tensor_copy(cs[:], pv[h][64:65, :])
                            csd[key].append(cs)
                        unorm[key] = [
                            sp.tile([64, 1024], F32, tag=f"un{h}", name=f"un{key}_{h}")
                            for h in range(2)
                        ]
                        for h in range(2):
                            if flush and h == 1:  # ScalarE is idle in the tail
                                nc.scalar.activation(unorm[key][h][:], pv[h][0:64, :], Copy)
                            else:
                                nc.vector.tensor_copy(unorm[key][h][:], pv[h][0:64, :])

                    for jt in range(16):
                        j0 = b * 2048 + jt * 128
                        s_h = [
                            sps.tile([128, 1024], F32, tag=f"s{h}", name=f"s{key}_{jt}_{h}")
                            for h in range(2)
                        ]
                        e_h = [
                            ep.tile([128, 1024], BF16, tag=f"e{h}", bufs=17,
                                    name=f"e{key}_{jt}_{h}")
                            for h in range(2)
                        ]
                        # h-major so h0's S+exp only gate on ACT_h0(jt-1):
                        # the two heads' ACTs ping-pong and ScalarE stays busy
                        for h in range(2):
                            for ih in range(2):
                                nc.tensor.matmul(
                                    s_h[h][:, ih * 512 : (ih + 1) * 512],
                                    ks[h][:, j0 : j0 + 128],
                                    qs[h][:, i0 + ih * 512 : i0 + (ih + 1) * 512],
                                    start=True,
                                    stop=True,
                                )
                            nc.scalar.activation(e_h[h][:], s_h[h][:], Exp, scale=SCALE)
                        e_pend.append(e_h)
                        pend.append(lambda jt=jt, f=emit_pv: f(jt))

                        # previous block's norm + out-projection, emitted
                        # before this block's pv allocations enter the rings
                        if prev is not None:
                            if jt in (5, 6):
                                emit_boundary(prev[0], prev[1], jt - 5)
                            elif jt == 7:
                                emit_boundary(prev[0], prev[1], 2)
                                emit_boundary(prev[0], prev[1], 3)
                            elif jt == 8:
                                emit_boundary(prev[0], prev[1], 4)
                                emit_boundary(prev[0], prev[1], 5)
                        tgt = TARGET_LAST if (b, ib) == blocks[-1] else TARGET
                        while len(pend) > tgt[jt]:
                            pend.pop(0)()
                    pend.append(emit_evac)
                    prev = (b, ib)

                # drain everything and flush the last block's norm + po
                flush = True
                for f in pend:
                    f()
                pend.clear()
                for step in range(6):
                    emit_boundary(prev[0], prev[1], step)

    nc.compile()
    return nc


_NC = None


def _get_nc():
    global _NC
    if _NC is None:
        _NC = build_nc()
    return _NC


def _gate(mask):
    """Exact jax fp32 gate: sigmoid(m) > 0.5 (matches reference rounding)."""
    mask = np.asarray(mask, dtype=np.float32)
    return (np.float32(1.0) / (np.float32(1.0) + np.exp(-mask))) > np.float32(0.5)


def make_in_maps(x, qkv_weight, qkv_weight_mask, out_weight, out_weight_mask):
    import ml_dtypes

    bf16 = ml_dtypes.bfloat16
    x = np.asarray(x, dtype=np.float32)
    wqkv = np.where(_gate(qkv_weight_mask), np.asarray(qkv_weight, np.float32), 0.0)
    wo = np.where(_gate(out_weight_mask), np.asarray(out_weight, np.float32), 0.0)

    xT = np.ascontiguousarray(x.reshape(T, DIM).T).astype(bf16)
    in_maps = []
    for c in range(NCORES):
        r0 = c * DV
        sl = slice(r0, r0 + DV)
        w_shard = np.concatenate(
            [wqkv[sl], wqkv[DIM + r0 : DIM + r0 + DV], wqkv[2 * DIM + r0 : 2 * DIM + r0 + DV]],
            axis=0,
        )  # [384, 1024] rows = (q | k | v) for this core's 2 heads
        in_maps.append(
            {
                "xT": xT,
                "wqkvT": np.ascontiguousarray(w_shard.T).astype(bf16),
                "woT": np.ascontiguousarray(wo[:, sl].T).astype(bf16),
            }
        )
    return in_maps


LAST_RESULTS = None  # BassKernelResults of the most recent run (for profiling)


def kernel(
    x,
    qkv_weight,
    qkv_weight_mask,
    out_weight,
    out_weight_mask,
    out_bias,
    out_bias_mask,
    _trace=False,
    _tmpdir=None,
):
    global LAST_RESULTS
    from concourse.bass_utils import run_bass_kernel_spmd

    nc = _get_nc()
    in_maps = make_in_maps(x, qkv_weight, qkv_weight_mask, out_weight, out_weight_mask)
    res = run_bass_kernel_spmd(
        nc, in_maps, list(range(NCORES)), trace=_trace, tmpdir=_tmpdir
    )
    LAST_RESULTS = res
    out = np.zeros((T, DIM), dtype=np.float32)
    for r in res.results:
        out += r["out"]
    out_bias = np.asarray(out_bias, dtype=np.float32)
    out += np.where(_gate(out_bias_mask), out_bias, np.float32(0.0))[None, :]
    return out.reshape(B, N, DIM)


# revision 46
# speedup vs baseline: 1.1742x; 1.0160x over previous
"""Trainium2 Bass kernel for nn_Attention_41704132444382.

Masked-linear QKV projection + 16-head attention + masked-linear output
projection, tensor-parallel over heads across 8 NeuronCores (2 heads/core).

Design (ScalarE exp streaming ~135us is the roofline; PE ~ matches it):
  - Host: gates both masked-linear weights (sigmoid(m)>0.5), transposes x,
    casts x / wqkv / wo to bf16 (wqkv/wo values are +-c, near-exact in bf16).
  - QKV: xq bf16 tiles; lhsT = gated wqkv bf16 (FWL weight loads); psum
    [128,512] chains; q is stored split-precision (hi/lo bf16 pair) so the
    S matmul recovers fp32-exact q in the otherwise-idle half of the PE
    array (k is duplicated); k/q evacuated by ScalarE, v by DVE.  V^T is
    PE-transposed (bf16) to v1/v2 [t, dv|1] tiles whose ones column makes
    the PV matmul emit the softmax denominator for free (M=65).
  - Attention per 1024-query block, h-offset pipeline: per key-tile jt,
    S as K=128 all-bf16 matmuls, one 1024-wide exp ACT per head
    (scale=1/32) -> e_h bf16; PV lags via a pend-queue drained to per-jt
    depth targets so block-boundary work (norm broadcast + out-projection)
    flows through the pv psum rings without stalling ScalarE.
  - Softmax denominators: pv row 64 -> [1,1024], PE-transpose chunks to
    partitions, DVE reciprocal, PE-transpose back, DMA row-gather, then
    a K=1 ones-matmul broadcast and one normalize tensor_tensor per head.
  - Output projection: lhsT = attnT bf16 (FWL), po pairs [128,1024] in
    the pv psum rings, evacs split DVE/ScalarE, DMA out from SBUF.
"""

import os
import sys

import numpy as np

sys.path.insert(0, "/opt/trn_rl_repo")

import concourse.bass as bass
import concourse.mybir as mybir
from concourse import bacc
from concourse.masks import make_identity
from concourse.tile import TileContext

DIM = 1024
HEADS = 16
B = 2
N = 2048
T = B * N  # 4096 flattened tokens
NCORES = 8
HPC = HEADS // NCORES  # 2 heads per core
DV = HPC * 64  # 128 head-dims per core
SCALE = DIM ** (-0.5)  # 1/32

F32 = mybir.dt.float32
F32R = mybir.dt.float32r
BF16 = mybir.dt.bfloat16

Copy = mybir.ActivationFunctionType.Copy
Exp = mybir.ActivationFunctionType.Exp
mult = mybir.AluOpType.mult


def build_nc():
    nc = bacc.Bacc("TRN2", target_bir_lowering=True)
    xT_d = nc.declare_dram_parameter("xT", [DIM, T], BF16, isOutput=False)
    wqkvT_d = nc.declare_dram_parameter("wqkvT", [DIM, 384], BF16, isOutput=False)
    woT_d = nc.declare_dram_parameter("woT", [DV, DIM], BF16, isOutput=False)
    out_d = nc.declare_dram_parameter("out", [T, DIM], F32, isOutput=True)

    with TileContext(nc) as tc:
        with tc.tile_pool(name="persist", bufs=1) as pp:
            # S runs as K=128 all-bf16 matmuls (FWL weight loads keep PE array
            # duty high -> HAM stays at 2.4 GHz) with split-precision q in the
            # otherwise-idle half of the array: qs rows 0-63 = bf16(q), rows
            # 64-127 = bf16(q - bf16(q)); ks duplicates k in both halves, so
            # k.T q accumulates the hi and lo products -> q is fp32-exact.
            qs = [pp.tile([128, T], BF16, name=f"qs{h}") for h in range(HPC)]
            ks = [pp.tile([128, T], BF16, name=f"ks{h}") for h in range(HPC)]
            v1 = pp.tile([128, 32 * 65], BF16)  # [t-part, (jt, dv|1)] head 0
            v2 = pp.tile([128, 32 * 65], BF16)  # head 1
            attnT = pp.tile([128, T], BF16)  # [dv-part, t] normalized
            wo_g = pp.tile([128, DIM], BF16)
            ident = pp.tile([128, 128], BF16)
            identf = pp.tile([128, 128], F32)
            ones1 = pp.tile([1, 64], F32R)

            make_identity(nc, ident[:])
            make_identity(nc, identf[:])
            ones_f = pp.tile([128, 64], F32)
            nc.vector.memset(ones_f[:], 1.0)
            nc.vector.tensor_copy(ones1[:], ones_f[0:1, :])
            ones32 = pp.tile([128, 32], BF16)
            nc.vector.tensor_copy(ones32[:], ones_f[:, 0:32])
            # ones column at slot 64 of each 65-wide block of v1/v2; V
            # evacuations only write cols 0..63 of each block.
            for vv in (v1, v2):
                nc.vector.tensor_copy(
                    vv[:].rearrange("p (j c) -> p j c", c=65)[:, :, 64:65],
                    ones32[:].rearrange("p (j c) -> p j c", c=1),
                )
            # preload the exp activation table while DMAs run
            junk = pp.tile([1, 32], F32)
            nc.vector.memset(junk[:], 0.0)
            junk2 = pp.tile([1, 32], F32)
            nc.scalar.activation(junk2[:], junk[:], Exp)

            # ---------- Phase 1: QKV projection (+ V^T transpose) ----------
            with (
                tc.tile_pool(name="ph1", bufs=1) as p1,
                tc.tile_pool(name="qkv_ps", bufs=4, space="PSUM") as qkps,
            ):
                wqkv_g = p1.tile([128, 8 * 384], BF16)  # [k-part, (kt, o)]
                nc.gpsimd.dma_start(
                    wqkv_g[:].rearrange("p (kt o) -> p kt o", kt=8),
                    wqkvT_d[:].rearrange("(kt p) o -> p kt o", p=128),
                )
                nc.gpsimd.dma_start(wo_g[:], woT_d[:])
                xq = [p1.tile([128, T], BF16, name=f"xq{i}") for i in range(8)]
                vT = p1.tile([128, T], BF16)
                # quarter 0 gets a dedicated queue + fine chunks so its
                # kt-chains start as soon as each slice lands
                for th in range(2):
                    for kt in range(8):
                        c0 = th * 512
                        nc.sync.dma_start(
                            xq[kt][:, c0 : c0 + 512],
                            xT_d[kt * 128 : (kt + 1) * 128, c0 : c0 + 512],
                        )
                dmae = [nc.gpsimd, nc.scalar]
                n = 0
                for q in range(1, 4):
                    for kt in range(8):
                        c0 = q * 1024
                        dmae[n % 2].dma_start(
                            xq[kt][:, c0 : c0 + 1024],
                            xT_d[kt * 128 : (kt + 1) * 128, c0 : c0 + 1024],
                        )
                        n += 1

                sub = mybir.AluOpType.subtract
                for q in range(4):
                    # v first so transposes can interleave with q/k matmuls
                    for ot in (2, 1, 0):
                        for th in range(2):
                            ps = qkps.tile([128, 512], F32, tag="qk")
                            for kt in range(8):
                                nc.tensor.matmul(
                                    ps[:],
                                    wqkv_g[:, kt * 384 + ot * 128 : kt * 384 + (ot + 1) * 128],
                                    xq[kt][:, q * 1024 + th * 512 : q * 1024 + (th + 1) * 512],
                                    start=(kt == 0),
                                    stop=(kt == 7),
                                )
                            col = q * 1024 + th * 512
                            cs_ = slice(col, col + 512)
                            if ot == 2:
                                nc.vector.tensor_copy(vT[:, cs_], ps[:])
                            elif ot == 0:  # q: hi = bf16(q), lo = q - hi
                                for hh in range(2):
                                    php = ps[hh * 64 : (hh + 1) * 64, :]
                                    nc.scalar.activation(qs[hh][0:64, cs_], php, Copy)
                                    nc.vector.tensor_tensor(
                                        qs[hh][64:128, cs_], php, qs[hh][0:64, cs_], sub
                                    )
                            else:  # k: duplicated into both array halves
                                for hh in range(2):
                                    php = ps[hh * 64 : (hh + 1) * 64, :]
                                    nc.scalar.activation(ks[hh][0:64, cs_], php, Copy)
                                    nc.vector.tensor_copy(ks[hh][64:128, cs_], ks[hh][0:64, cs_])
                    for tj in range(8):  # V^T -> v1/v2 for this quarter
                        jt = q * 8 + tj
                        ptv = qkps.tile([128, 128], BF16, tag="vt", bufs=2)
                        nc.tensor.transpose(
                            ptv[:], vT[:, jt * 128 : (jt + 1) * 128], ident[:]
                        )
                        nc.vector.tensor_copy(v1[:, jt * 65 : jt * 65 + 64], ptv[:, 0:64])
                        nc.vector.tensor_copy(v2[:, jt * 65 : jt * 65 + 64], ptv[:, 64:128])

            # ---------- Phase 2: attention ----------
            with (
                tc.tile_pool(name="esb", bufs=1) as ep,
                tc.tile_pool(name="small", bufs=1) as sp,
                tc.tile_pool(name="osb", bufs=1) as osp,
                tc.tile_pool(name="s_ps", bufs=1, space="PSUM") as sps,
                tc.tile_pool(name="pv_ps", bufs=1, space="PSUM") as pvps,
            ):
                blocks = [(b, ib) for b in range(B) for ib in range(2)]

                def emit_boundary(pb, pib, step):
                    """Norm + out-projection for block (pb, pib), interleaved
                    into the next block's jt loop (or flushed at the end).
                    step 0: denominator transposes + reciprocal; 1: broadcast
                    + normalize; 2..5: two po pairs each."""
                    i0 = pb * 2048 + pib * 1024
                    key = f"{pb}_{pib}"
                    if step == 0:
                        # colsum rows -> partitions, reciprocal on 128 lanes,
                        # transpose back, DMA row-gather to [1, 1024]
                        pt = pvps.tile([128, 16], F32, tag="pv0", name=f"pt{key}")
                        for h in range(2):
                            for blk in range(8):
                                c = h * 8 + blk
                                nc.tensor.transpose(
                                    pt[:, c : c + 1],
                                    csd[key][h][0:1, blk * 128 : (blk + 1) * 128],
                                    identf[0:1, 0:1],
                                )
                        rT = sp.tile([128, 16], F32, tag="rT", name=f"rT{key}")
                        nc.vector.reciprocal(rT[:], pt[:])
                        pr = pvps.tile([16, 128], F32, tag="pv1", name=f"pr{key}")
                        nc.tensor.transpose(pr[:], rT[:], identf[:])
                        prs = sp.tile([16, 128], F32R, tag="prs", name=f"prs{key}")
                        nc.vector.tensor_copy(prs[:], pr[:])
                        r2 = [
                            sp.tile([1, 1024], F32R, tag=f"r{h}", name=f"r{key}_{h}")
                            for h in range(2)
                        ]
                        for h in range(2):
                            nc.sync.dma_start(r2[h][0:1, :], prs[h * 8 : (h + 1) * 8, :])
                        rcp[key] = r2
                    elif step == 1:
                        for h in range(2):
                            rbc = pvps.tile(
                                [64, 1024], F32, tag=f"pv{h}", name=f"rbc{key}_{h}"
                            )
                            for ih in range(2):
                                nc.tensor.matmul(
                                    rbc[:, ih * 512 : (ih + 1) * 512],
                                    ones1[:],
                                    rcp[key][h][0:1, ih * 512 : (ih + 1) * 512],
                                    start=True,
                                    stop=True,
                                )
                            rbs = sp.tile([64, 1024], F32, tag=f"rbs{h}", name=f"rbs{key}_{h}")
                            nc.vector.tensor_copy(rbs[:], rbc[:])
                            nc.vector.tensor_tensor(
                                attnT[h * 64 : (h + 1) * 64, i0 : i0 + 1024],
                                unorm[key][h][:],
                                rbs[:],
                                mult,
                            )
                    else:
                        for k in range(2):
                            tg = (step - 2) * 2 + k
                            row = i0 + tg * 128
                            po = pvps.tile(
                                [128, 1024], F32, tag=f"pv{k}", name=f"po{key}_{tg}"
                            )
                            for oh in range(2):
                                nc.tensor.matmul(
                                    po[:, oh * 512 : (oh + 1) * 512],
                                    attnT[:, row : row + 128],
                                    wo_g[:, oh * 512 : (oh + 1) * 512],
                                    start=True,
                                    stop=True,
                                )
                            ob = osp.tile(
                                [128, 1024], F32, tag="ob", bufs=6, name=f"ob{key}_{tg}"
                            )
                            # in the tail (flush) ScalarE is idle: split evacs
                            # (mid-run its strict-FIFO queue must stay clear
                            # for exps -- a waiting Copy head-of-line blocks)
                            if flush and tg % 2 == 1:
                                nc.scalar.activation(ob[:], po[:], Copy)
                            else:
                                nc.vector.tensor_copy(ob[:], po[:])
                            dmae = nc.sync if tg % 2 == 0 else nc.gpsimd
                            dmae.dma_start(out_d[row : row + 128, :], ob[:])

                unorm = {}
                rcp = {}
                csd = {}
                flush = False
                pend = []  # closures: lagged PV groups + block-end evacuations
                # drain to a per-jt target queue depth: PE slack per jt fits
                # ~1.2 PV groups, so each block's PV tail spills into the next
                # block's early key-tiles (norm at jt5-6, po pairs at jt7-8);
                # the targets keep pv allocations at jt9, after the previous
                # block's norm/po tiles in the psum rings.  The last block
                # drains aggressively so the tail chain starts early.
                TARGET = [9, 8, 7, 6, 5, 6, 7, 8, 9, 9, 9, 9, 9, 9, 9, 9]
                TARGET_LAST = [9, 8, 7, 6, 5, 6, 7, 8, 9, 9, 8, 6, 4, 2, 1, 0]
                prev = None
                for b, ib in blocks:
                    key = f"{b}_{ib}"
                    i0 = b * 2048 + ib * 1024
                    # allocated lazily at the first emit_pv so the pv-ring
                    # order is: prev block's pv -> prev's rbc/po -> ours
                    pv = []
                    e_pend = []

                    def emit_pv(jt, key=key, b=b, pv=pv, e_pend=e_pend):
                        if not pv:
                            pv.extend(
                                pvps.tile([65, 1024], F32, tag=f"pv{h}", name=f"pv{key}_{h}")
                                for h in range(2)
                            )
                        eh = e_pend.pop(0)
                        jv = (b * 16 + jt) * 65
                        for h, vv in enumerate((v1, v2)):
                            for ih in range(2):
                                nc.tensor.matmul(
                                    pv[h][:, ih * 512 : (ih + 1) * 512],
                                    vv[:, jv : jv + 65],
                                    eh[h][:, ih * 512 : (ih + 1) * 512],
                                    start=(jt == 0),
                                    stop=(jt == 15),
                                )

                    def emit_evac(key=key, pv=pv):
                        # denominator rows + unnormalized attn-out to SBUF;
                        # frees the pv psum banks (norm continues next block)
                        csd[key] = []
                        for h in range(2):
                            cs = sp.tile([1, 1024], F32, tag=f"cs{h}", name=f"cs{key}_{h}")
                            nc.vector.tensor_copy(cs[:], pv[h][64:65, :])
                            csd[key].append(cs)
                        unorm[key] = [
                            sp.tile([64, 1024], F32, tag=f"un{h}", name=f"un{key}_{h}")
                            for h in range(2)
                        ]
                        for h in range(2):
                            if flush and h == 1:  # ScalarE is idle in the tail
                                nc.scalar.activation(unorm[key][h][:], pv[h][0:64, :], Copy)
                            else:
                                nc.vector.tensor_copy(unorm[key][h][:], pv[h][0:64, :])

                    for jt in range(16):
                        j0 = b * 2048 + jt * 128
                        s_h = [
                            sps.tile([128, 1024], F32, tag=f"s{h}", name=f"s{key}_{jt}_{h}")
                            for h in range(2)
                        ]
                        e_h = [
                            ep.tile([128, 1024], BF16, tag=f"e{h}", bufs=17,
                                    name=f"e{key}_{jt}_{h}")
                            for h in range(2)
                        ]
                        # h-major so h0's S+exp only gate on ACT_h0(jt-1):
                        # the two heads' ACTs ping-pong and ScalarE stays busy
                        for h in range(2):
                            for ih in range(2):
                                nc.tensor.matmul(
                                    s_h[h][:, ih * 512 : (ih + 1) * 512],
                                    ks[h][:, j0 : j0 + 128],
                                    qs[h][:, i0 + ih * 512 : i0 + (ih + 1) * 512],
                                    start=True,
                                    stop=True,
                                )
                            nc.scalar.activation(e_h[h][:], s_h[h][:], Exp, scale=SCALE)
                        e_pend.append(e_h)
                        pend.append(lambda jt=jt, f=emit_pv: f(jt))

                        # previous block's norm + out-projection, emitted
                        # before this block's pv allocations enter the rings
                        if prev is not None:
                            if jt in (5, 6):
                                emit_boundary(prev[0], prev[1], jt - 5)
                            elif jt == 7:
                                emit_boundary(prev[0], prev[1], 2)
                                emit_boundary(prev[0], prev[1], 3)
                            elif jt == 8:
                                emit_boundary(prev[0], prev[1], 4)
                                emit_boundary(prev[0], prev[1], 5)
                        tgt = TARGET_LAST if (b, ib) == blocks[-1] else TARGET
                        while len(pend) > tgt[jt]:
                            pend.pop(0)()
                    pend.append(emit_evac)
                    prev = (b, ib)

                # drain everything and flush the last block's norm + po
                flush = True
                for f in pend:
                    f()
                pend.clear()
                for step in range(6):
                    emit_boundary(prev[0], prev[1], step)

    nc.compile()
    return nc


_NC = None


def _get_nc():
    global _NC
    if _NC is None:
        _NC = build_nc()
    return _NC


def _gate(mask):
    """Exact jax fp32 gate: sigmoid(m) > 0.5 (matches reference rounding)."""
    mask = np.asarray(mask, dtype=np.float32)
    return (np.float32(1.0) / (np.float32(1.0) + np.exp(-mask))) > np.float32(0.5)


def make_in_maps(x, qkv_weight, qkv_weight_mask, out_weight, out_weight_mask):
    import ml_dtypes

    bf16 = ml_dtypes.bfloat16
    x = np.asarray(x, dtype=np.float32)
    wqkv = np.where(_gate(qkv_weight_mask), np.asarray(qkv_weight, np.float32), 0.0)
    wo = np.where(_gate(out_weight_mask), np.asarray(out_weight, np.float32), 0.0)

    xT = np.ascontiguousarray(x.reshape(T, DIM).T).astype(bf16)
    in_maps = []
    for c in range(NCORES):
        r0 = c * DV
        sl = slice(r0, r0 + DV)
        w_shard = np.concatenate(
            [wqkv[sl], wqkv[DIM + r0 : DIM + r0 + DV], wqkv[2 * DIM + r0 : 2 * DIM + r0 + DV]],
            axis=0,
        )  # [384, 1024] rows = (q | k | v) for this core's 2 heads
        in_maps.append(
            {
                "xT": xT,
                "wqkvT": np.ascontiguousarray(w_shard.T).astype(bf16),
                "woT": np.ascontiguousarray(wo[:, sl].T).astype(bf16),
            }
        )
    return in_maps


LAST_RESULTS = None  # BassKernelResults of the most recent run (for profiling)


def kernel(
    x,
    qkv_weight,
    qkv_weight_mask,
    out_weight,
    out_weight_mask,
    out_bias,
    out_bias_mask,
    _trace=False,
    _tmpdir=None,
):
    global LAST_RESULTS
    from concourse.bass_utils import run_bass_kernel_spmd

    nc = _get_nc()
    in_maps = make_in_maps(x, qkv_weight, qkv_weight_mask, out_weight, out_weight_mask)
    res = run_bass_kernel_spmd(
        nc, in_maps, list(range(NCORES)), trace=_trace, tmpdir=_tmpdir
    )
    LAST_RESULTS = res
    out = np.zeros((T, DIM), dtype=np.float32)
    for r in res.results:
        out += r["out"]
    out_bias = np.asarray(out_bias, dtype=np.float32)
    out += np.where(_gate(out_bias_mask), out_bias, np.float32(0.0))[None, :]
    return out.reshape(B, N, DIM)


# revision 47
# speedup vs baseline: 1.1760x; 1.0015x over previous
"""Trainium2 Bass kernel for nn_Attention_41704132444382.

Masked-linear QKV projection + 16-head attention + masked-linear output
projection, tensor-parallel over heads across 8 NeuronCores (2 heads/core).

Design (ScalarE exp streaming ~135us is the roofline; PE ~ matches it):
  - Host: gates both masked-linear weights (sigmoid(m)>0.5), transposes x,
    casts x / wqkv / wo to bf16 (wqkv/wo values are +-c, near-exact in bf16).
  - QKV: xq bf16 tiles; lhsT = gated wqkv bf16 (FWL weight loads); psum
    [128,512] chains; q is stored split-precision (hi/lo bf16 pair) so the
    S matmul recovers fp32-exact q in the otherwise-idle half of the PE
    array (k is duplicated); k/q evacuated by ScalarE, v by DVE.  V^T is
    PE-transposed (bf16) to v1/v2 [t, dv|1] tiles whose ones column makes
    the PV matmul emit the softmax denominator for free (M=65).
  - Attention per 1024-query block, h-offset pipeline: per key-tile jt,
    S as K=128 all-bf16 matmuls, one 1024-wide exp ACT per head
    (scale=1/32) -> e_h bf16; PV lags via a pend-queue drained to per-jt
    depth targets so block-boundary work (norm broadcast + out-projection)
    flows through the pv psum rings without stalling ScalarE.
  - Softmax denominators: pv row 64 -> [1,1024], PE-transpose chunks to
    partitions, DVE reciprocal, PE-transpose back, DMA row-gather, then
    a K=1 ones-matmul broadcast and one normalize tensor_tensor per head.
  - Output projection: lhsT = attnT bf16 (FWL), po pairs [128,1024] in
    the pv psum rings, evacs split DVE/ScalarE, DMA out from SBUF.
"""

import os
import sys

import numpy as np

sys.path.insert(0, "/opt/trn_rl_repo")

import concourse.bass as bass
import concourse.mybir as mybir
from concourse import bacc
from concourse.masks import make_identity
from concourse.tile import TileContext

DIM = 1024
HEADS = 16
B = 2
N = 2048
T = B * N  # 4096 flattened tokens
NCORES = 8
HPC = HEADS // NCORES  # 2 heads per core
DV = HPC * 64  # 128 head-dims per core
SCALE = DIM ** (-0.5)  # 1/32

F32 = mybir.dt.float32
F32R = mybir.dt.float32r
BF16 = mybir.dt.bfloat16

Copy = mybir.ActivationFunctionType.Copy
Exp = mybir.ActivationFunctionType.Exp
mult = mybir.AluOpType.mult


def build_nc():
    nc = bacc.Bacc("TRN2", target_bir_lowering=True)
    xT_d = nc.declare_dram_parameter("xT", [DIM, T], BF16, isOutput=False)
    wqkvT_d = nc.declare_dram_parameter("wqkvT", [DIM, 384], BF16, isOutput=False)
    woT_d = nc.declare_dram_parameter("woT", [DV, DIM], BF16, isOutput=False)
    out_d = nc.declare_dram_parameter("out", [T, DIM], F32, isOutput=True)

    with TileContext(nc) as tc:
        with tc.tile_pool(name="persist", bufs=1) as pp:
            # S runs as K=128 all-bf16 matmuls (FWL weight loads keep PE array
            # duty high -> HAM stays at 2.4 GHz) with split-precision q in the
            # otherwise-idle half of the array: qs rows 0-63 = bf16(q), rows
            # 64-127 = bf16(q - bf16(q)); ks duplicates k in both halves, so
            # k.T q accumulates the hi and lo products -> q is fp32-exact.
            qs = [pp.tile([128, T], BF16, name=f"qs{h}") for h in range(HPC)]
            ks = [pp.tile([128, T], BF16, name=f"ks{h}") for h in range(HPC)]
            v1 = pp.tile([128, 32 * 65], BF16)  # [t-part, (jt, dv|1)] head 0
            v2 = pp.tile([128, 32 * 65], BF16)  # head 1
            attnT = pp.tile([128, T], BF16)  # [dv-part, t] normalized
            wo_g = pp.tile([128, DIM], BF16)
            ident = pp.tile([128, 128], BF16)
            identf = pp.tile([128, 128], F32)
            ones1 = pp.tile([1, 64], F32R)

            make_identity(nc, ident[:])
            make_identity(nc, identf[:])
            ones_f = pp.tile([128, 64], F32)
            nc.vector.memset(ones_f[:], 1.0)
            nc.vector.tensor_copy(ones1[:], ones_f[0:1, :])
            ones32 = pp.tile([128, 32], BF16)
            nc.vector.tensor_copy(ones32[:], ones_f[:, 0:32])
            # ones column at slot 64 of each 65-wide block of v1/v2; V
            # evacuations only write cols 0..63 of each block.
            for vv in (v1, v2):
                nc.vector.tensor_copy(
                    vv[:].rearrange("p (j c) -> p j c", c=65)[:, :, 64:65],
                    ones32[:].rearrange("p (j c) -> p j c", c=1),
                )
            # preload the exp activation table while DMAs run
            junk = pp.tile([1, 32], F32)
            nc.vector.memset(junk[:], 0.0)
            junk2 = pp.tile([1, 32], F32)
            nc.scalar.activation(junk2[:], junk[:], Exp)

            # ---------- Phase 1: QKV projection (+ V^T transpose) ----------
            with (
                tc.tile_pool(name="ph1", bufs=1) as p1,
                tc.tile_pool(name="qkv_ps", bufs=4, space="PSUM") as qkps,
            ):
                wqkv_g = p1.tile([128, 8 * 384], BF16)  # [k-part, (kt, o)]
                nc.gpsimd.dma_start(
                    wqkv_g[:].rearrange("p (kt o) -> p kt o", kt=8),
                    wqkvT_d[:].rearrange("(kt p) o -> p kt o", p=128),
                )
                nc.gpsimd.dma_start(wo_g[:], woT_d[:])
                xq = [p1.tile([128, T], BF16, name=f"xq{i}") for i in range(8)]
                vT = p1.tile([128, T], BF16)
                # quarter 0 gets a dedicated queue + fine chunks so its
                # kt-chains start as soon as each slice lands
                for th in range(2):
                    for kt in range(8):
                        c0 = th * 512
                        nc.sync.dma_start(
                            xq[kt][:, c0 : c0 + 512],
                            xT_d[kt * 128 : (kt + 1) * 128, c0 : c0 + 512],
                        )
                dmae = [nc.sync, nc.gpsimd, nc.scalar]
                n = 0
                for q in range(1, 4):
                    for kt in range(8):
                        c0 = q * 1024
                        dmae[n % 3].dma_start(
                            xq[kt][:, c0 : c0 + 1024],
                            xT_d[kt * 128 : (kt + 1) * 128, c0 : c0 + 1024],
                        )
                        n += 1

                sub = mybir.AluOpType.subtract
                for q in range(4):
                    # v first so transposes can interleave with q/k matmuls
                    for ot in (2, 1, 0):
                        for th in range(2):
                            ps = qkps.tile([128, 512], F32, tag="qk")
                            for kt in range(8):
                                nc.tensor.matmul(
                                    ps[:],
                                    wqkv_g[:, kt * 384 + ot * 128 : kt * 384 + (ot + 1) * 128],
                                    xq[kt][:, q * 1024 + th * 512 : q * 1024 + (th + 1) * 512],
                                    start=(kt == 0),
                                    stop=(kt == 7),
                                )
                            col = q * 1024 + th * 512
                            cs_ = slice(col, col + 512)
                            if ot == 2:
                                nc.vector.tensor_copy(vT[:, cs_], ps[:])
                            elif ot == 0:  # q: hi = bf16(q), lo = q - hi
                                for hh in range(2):
                                    php = ps[hh * 64 : (hh + 1) * 64, :]
                                    nc.scalar.activation(qs[hh][0:64, cs_], php, Copy)
                                    nc.vector.tensor_tensor(
                                        qs[hh][64:128, cs_], php, qs[hh][0:64, cs_], sub
                                    )
                            else:  # k: duplicated into both array halves
                                for hh in range(2):
                                    php = ps[hh * 64 : (hh + 1) * 64, :]
                                    nc.scalar.activation(ks[hh][0:64, cs_], php, Copy)
                                    nc.vector.tensor_copy(ks[hh][64:128, cs_], ks[hh][0:64, cs_])
                    for tj in range(8):  # V^T -> v1/v2 for this quarter
                        jt = q * 8 + tj
                        ptv = qkps.tile([128, 128], BF16, tag="vt", bufs=2)
                        nc.tensor.transpose(
                            ptv[:], vT[:, jt * 128 : (jt + 1) * 128], ident[:]
                        )
                        nc.vector.tensor_copy(v1[:, jt * 65 : jt * 65 + 64], ptv[:, 0:64])
                        nc.vector.tensor_copy(v2[:, jt * 65 : jt * 65 + 64], ptv[:, 64:128])

            # ---------- Phase 2: attention ----------
            with (
                tc.tile_pool(name="esb", bufs=1) as ep,
                tc.tile_pool(name="small", bufs=1) as sp,
                tc.tile_pool(name="osb", bufs=1) as osp,
                tc.tile_pool(name="s_ps", bufs=1, space="PSUM") as sps,
                tc.tile_pool(name="pv_ps", bufs=1, space="PSUM") as pvps,
            ):
                blocks = [(b, ib) for b in range(B) for ib in range(2)]

                def emit_boundary(pb, pib, step):
                    """Norm + out-projection for block (pb, pib), interleaved
                    into the next block's jt loop (or flushed at the end).
                    step 0: denominator transposes + reciprocal; 1: broadcast
                    + normalize; 2..5: two po pairs each."""
                    i0 = pb * 2048 + pib * 1024
                    key = f"{pb}_{pib}"
                    if step == 0:
                        # colsum rows -> partitions, reciprocal on 128 lanes,
                        # transpose back, DMA row-gather to [1, 1024]
                        pt = pvps.tile([128, 16], F32, tag="pv0", name=f"pt{key}")
                        for h in range(2):
                            for blk in range(8):
                                c = h * 8 + blk
                                nc.tensor.transpose(
                                    pt[:, c : c + 1],
                                    csd[key][h][0:1, blk * 128 : (blk + 1) * 128],
                                    identf[0:1, 0:1],
                                )
                        rT = sp.tile([128, 16], F32, tag="rT", name=f"rT{key}")
                        nc.vector.reciprocal(rT[:], pt[:])
                        pr = pvps.tile([16, 128], F32, tag="pv1", name=f"pr{key}")
                        nc.tensor.transpose(pr[:], rT[:], identf[:])
                        prs = sp.tile([16, 128], F32R, tag="prs", name=f"prs{key}")
                        nc.vector.tensor_copy(prs[:], pr[:])
                        r2 = [
                            sp.tile([1, 1024], F32R, tag=f"r{h}", name=f"r{key}_{h}")
                            for h in range(2)
                        ]
                        for h in range(2):
                            nc.sync.dma_start(r2[h][0:1, :], prs[h * 8 : (h + 1) * 8, :])
                        rcp[key] = r2
                    elif step == 1:
                        for h in range(2):
                            rbc = pvps.tile(
                                [64, 1024], F32, tag=f"pv{h}", name=f"rbc{key}_{h}"
                            )
                            for ih in range(2):
                                nc.tensor.matmul(
                                    rbc[:, ih * 512 : (ih + 1) * 512],
                                    ones1[:],
                                    rcp[key][h][0:1, ih * 512 : (ih + 1) * 512],
                                    start=True,
                                    stop=True,
                                )
                            rbs = sp.tile([64, 1024], F32, tag=f"rbs{h}", name=f"rbs{key}_{h}")
                            nc.vector.tensor_copy(rbs[:], rbc[:])
                            nc.vector.tensor_tensor(
                                attnT[h * 64 : (h + 1) * 64, i0 : i0 + 1024],
                                unorm[key][h][:],
                                rbs[:],
                                mult,
                            )
                    else:
                        for k in range(2):
                            tg = (step - 2) * 2 + k
                            row = i0 + tg * 128
                            po = pvps.tile(
                                [128, 1024], F32, tag=f"pv{k}", name=f"po{key}_{tg}"
                            )
                            for oh in range(2):
                                nc.tensor.matmul(
                                    po[:, oh * 512 : (oh + 1) * 512],
                                    attnT[:, row : row + 128],
                                    wo_g[:, oh * 512 : (oh + 1) * 512],
                                    start=True,
                                    stop=True,
                                )
                            ob = osp.tile(
                                [128, 1024], F32, tag="ob", bufs=6, name=f"ob{key}_{tg}"
                            )
                            # in the tail (flush) ScalarE is idle: split evacs
                            # (mid-run its strict-FIFO queue must stay clear
                            # for exps -- a waiting Copy head-of-line blocks)
                            if flush and tg % 2 == 1:
                                nc.scalar.activation(ob[:], po[:], Copy)
                            else:
                                nc.vector.tensor_copy(ob[:], po[:])
                            dmae = nc.sync if tg % 2 == 0 else nc.gpsimd
                            dmae.dma_start(out_d[row : row + 128, :], ob[:])

                unorm = {}
                rcp = {}
                csd = {}
                flush = False
                pend = []  # closures: lagged PV groups + block-end evacuations
                # drain to a per-jt target queue depth: PE slack per jt fits
                # ~1.2 PV groups, so each block's PV tail spills into the next
                # block's early key-tiles (norm at jt5-6, po pairs at jt7-8);
                # the targets keep pv allocations at jt9, after the previous
                # block's norm/po tiles in the psum rings.  The last block
                # drains aggressively so the tail chain starts early.
                TARGET = [9, 8, 7, 6, 5, 6, 7, 8, 9, 9, 9, 9, 9, 9, 9, 9]
                TARGET_LAST = [9, 8, 7, 6, 5, 6, 7, 8, 9, 9, 8, 6, 4, 2, 1, 0]
                prev = None
                for b, ib in blocks:
                    key = f"{b}_{ib}"
                    i0 = b * 2048 + ib * 1024
                    # allocated lazily at the first emit_pv so the pv-ring
                    # order is: prev block's pv -> prev's rbc/po -> ours
                    pv = []
                    e_pend = []

                    def emit_pv(jt, key=key, b=b, pv=pv, e_pend=e_pend):
                        if not pv:
                            pv.extend(
                                pvps.tile([65, 1024], F32, tag=f"pv{h}", name=f"pv{key}_{h}")
                                for h in range(2)
                            )
                        eh = e_pend.pop(0)
                        jv = (b * 16 + jt) * 65
                        for h, vv in enumerate((v1, v2)):
                            for ih in range(2):
                                nc.tensor.matmul(
                                    pv[h][:, ih * 512 : (ih + 1) * 512],
                                    vv[:, jv : jv + 65],
                                    eh[h][:, ih * 512 : (ih + 1) * 512],
                                    start=(jt == 0),
                                    stop=(jt == 15),
                                )

                    def emit_evac(key=key, pv=pv):
                        # denominator rows + unnormalized attn-out to SBUF;
                        # frees the pv psum banks (norm continues next block)
                        csd[key] = []
                        for h in range(2):
                            cs = sp.tile([1, 1024], F32, tag=f"cs{h}", name=f"cs{key}_{h}")
                            nc.vector.tensor_copy(cs[:], pv[h][64:65, :])
                            csd[key].append(cs)
                        unorm[key] = [
                            sp.tile([64, 1024], F32, tag=f"un{h}", name=f"un{key}_{h}")
                            for h in range(2)
                        ]
                        for h in range(2):
                            if flush and h == 1:  # ScalarE is idle in the tail
                                nc.scalar.activation(unorm[key][h][:], pv[h][0:64, :], Copy)
                            else:
                                nc.vector.tensor_copy(unorm[key][h][:], pv[h][0:64, :])

                    for jt in range(16):
                        j0 = b * 2048 + jt * 128
                        s_h = [
                            sps.tile([128, 1024], F32, tag=f"s{h}", name=f"s{key}_{jt}_{h}")
                            for h in range(2)
                        ]
                        e_h = [
                            ep.tile([128, 1024], BF16, tag=f"e{h}", bufs=17,
                                    name=f"e{key}_{jt}_{h}")
                            for h in range(2)
                        ]
                        # h-major so h0's S+exp only gate on ACT_h0(jt-1):
                        # the two heads' ACTs ping-pong and ScalarE stays busy
                        for h in range(2):
                            for ih in range(2):
                                nc.tensor.matmul(
                                    s_h[h][:, ih * 512 : (ih + 1) * 512],
                                    ks[h][:, j0 : j0 + 128],
                                    qs[h][:, i0 + ih * 512 : i0 + (ih + 1) * 512],
                                    start=True,
                                    stop=True,
                                )
                            nc.scalar.activation(e_h[h][:], s_h[h][:], Exp, scale=SCALE)
                        e_pend.append(e_h)
                        pend.append(lambda jt=jt, f=emit_pv: f(jt))

                        # previous block's norm + out-projection, emitted
                        # before this block's pv allocations enter the rings
                        if prev is not None:
                            if jt in (5, 6):
                                emit_boundary(prev[0], prev[1], jt - 5)
                            elif jt == 7:
                                emit_boundary(prev[0], prev[1], 2)
                                emit_boundary(prev[0], prev[1], 3)
                            elif jt == 8:
                                emit_boundary(prev[0], prev[1], 4)
                                emit_boundary(prev[0], prev[1], 5)
                        tgt = TARGET_LAST if (b, ib) == blocks[-1] else TARGET
                        while len(pend) > tgt[jt]:
                            pend.pop(0)()
                    pend.append(emit_evac)
                    prev = (b, ib)

                # drain everything and flush the last block's norm + po
                flush = True
                for f in pend:
                    f()
                pend.clear()
                for step in range(6):
                    emit_boundary(prev[0], prev[1], step)

    nc.compile()
    return nc


_NC = None


def _get_nc():
    global _NC
    if _NC is None:
        _NC = build_nc()
    return _NC


def _gate(mask):
    """Exact jax fp32 gate: sigmoid(m) > 0.5 (matches reference rounding)."""
    mask = np.asarray(mask, dtype=np.float32)
    return (np.float32(1.0) / (np.float32(1.0) + np.exp(-mask))) > np.float32(0.5)


def make_in_maps(x, qkv_weight, qkv_weight_mask, out_weight, out_weight_mask):
    import ml_dtypes

    bf16 = ml_dtypes.bfloat16
    x = np.asarray(x, dtype=np.float32)
    wqkv = np.where(_gate(qkv_weight_mask), np.asarray(qkv_weight, np.float32), 0.0)
    wo = np.where(_gate(out_weight_mask), np.asarray(out_weight, np.float32), 0.0)

    xT = np.ascontiguousarray(x.reshape(T, DIM).T).astype(bf16)
    in_maps = []
    for c in range(NCORES):
        r0 = c * DV
        sl = slice(r0, r0 + DV)
        w_shard = np.concatenate(
            [wqkv[sl], wqkv[DIM + r0 : DIM + r0 + DV], wqkv[2 * DIM + r0 : 2 * DIM + r0 + DV]],
            axis=0,
        )  # [384, 1024] rows = (q | k | v) for this core's 2 heads
        in_maps.append(
            {
                "xT": xT,
                "wqkvT": np.ascontiguousarray(w_shard.T).astype(bf16),
                "woT": np.ascontiguousarray(wo[:, sl].T).astype(bf16),
            }
        )
    return in_maps


LAST_RESULTS = None  # BassKernelResults of the most recent run (for profiling)


def kernel(
    x,
    qkv_weight,
    qkv_weight_mask,
    out_weight,
    out_weight_mask,
    out_bias,
    out_bias_mask,
    _trace=False,
    _tmpdir=None,
):
    global LAST_RESULTS
    from concourse.bass_utils import run_bass_kernel_spmd

    nc = _get_nc()
    in_maps = make_in_maps(x, qkv_weight, qkv_weight_mask, out_weight, out_weight_mask)
    res = run_bass_kernel_spmd(
        nc, in_maps, list(range(NCORES)), trace=_trace, tmpdir=_tmpdir
    )
    LAST_RESULTS = res
    out = np.zeros((T, DIM), dtype=np.float32)
    for r in res.results:
        out += r["out"]
    out_bias = np.asarray(out_bias, dtype=np.float32)
    out += np.where(_gate(out_bias_mask), out_bias, np.float32(0.0))[None, :]
    return out.reshape(B, N, DIM)


# revision 48
# speedup vs baseline: 1.1859x; 1.0084x over previous
"""Trainium2 Bass kernel for nn_Attention_41704132444382.

Masked-linear QKV projection + 16-head attention + masked-linear output
projection, tensor-parallel over heads across 8 NeuronCores (2 heads/core).

Design (ScalarE exp streaming ~135us is the roofline; PE ~ matches it):
  - Host: gates both masked-linear weights (sigmoid(m)>0.5), transposes x,
    casts x / wqkv / wo to bf16 (wqkv/wo values are +-c, near-exact in bf16).
  - QKV: xq bf16 tiles; lhsT = gated wqkv bf16 (FWL weight loads); psum
    [128,512] chains; q is stored split-precision (hi/lo bf16 pair) so the
    S matmul recovers fp32-exact q in the otherwise-idle half of the PE
    array (k is duplicated); k/q evacuated by ScalarE, v by DVE.  V^T is
    PE-transposed (bf16) to v1/v2 [t, dv|1] tiles whose ones column makes
    the PV matmul emit the softmax denominator for free (M=65).
  - Attention per 1024-query block, h-offset pipeline: per key-tile jt,
    S as K=128 all-bf16 matmuls, one 1024-wide exp ACT per head
    (scale=1/32) -> e_h bf16; PV lags via a pend-queue drained to per-jt
    depth targets so block-boundary work (norm broadcast + out-projection)
    flows through the pv psum rings without stalling ScalarE.
  - Softmax denominators: pv row 64 -> [1,1024], PE-transpose chunks to
    partitions, DVE reciprocal, PE-transpose back, DMA row-gather, then
    a K=1 ones-matmul broadcast and one normalize tensor_tensor per head.
  - Output projection: lhsT = attnT bf16 (FWL), po pairs [128,1024] in
    the pv psum rings, evacs split DVE/ScalarE, DMA out from SBUF.
"""

import os
import sys

import numpy as np

sys.path.insert(0, "/opt/trn_rl_repo")

import concourse.bass as bass
import concourse.mybir as mybir
from concourse import bacc
from concourse.masks import make_identity
from concourse.tile import TileContext

DIM = 1024
HEADS = 16
B = 2
N = 2048
T = B * N  # 4096 flattened tokens
NCORES = 8
HPC = HEADS // NCORES  # 2 heads per core
DV = HPC * 64  # 128 head-dims per core
SCALE = DIM ** (-0.5)  # 1/32

F32 = mybir.dt.float32
F32R = mybir.dt.float32r
BF16 = mybir.dt.bfloat16

Copy = mybir.ActivationFunctionType.Copy
Exp = mybir.ActivationFunctionType.Exp
mult = mybir.AluOpType.mult


def build_nc():
    nc = bacc.Bacc("TRN2", target_bir_lowering=True)
    xT_d = nc.declare_dram_parameter("xT", [DIM, T], BF16, isOutput=False)
    wqkvT_d = nc.declare_dram_parameter("wqkvT", [DIM, 384], BF16, isOutput=False)
    woT_d = nc.declare_dram_parameter("woT", [DV, DIM], BF16, isOutput=False)
    out_d = nc.declare_dram_parameter("out", [T, DIM], F32, isOutput=True)

    with TileContext(nc) as tc:
        with tc.tile_pool(name="persist", bufs=1) as pp:
            # S runs as K=128 all-bf16 matmuls (FWL weight loads keep PE array
            # duty high -> HAM stays at 2.4 GHz) with split-precision q in the
            # otherwise-idle half of the array: qs rows 0-63 = bf16(q), rows
            # 64-127 = bf16(q - bf16(q)); ks duplicates k in both halves, so
            # k.T q accumulates the hi and lo products -> q is fp32-exact.
            qs = [pp.tile([128, T], BF16, name=f"qs{h}") for h in range(HPC)]
            ks = [pp.tile([128, T], BF16, name=f"ks{h}") for h in range(HPC)]
            v1 = pp.tile([128, 32 * 65], BF16)  # [t-part, (jt, dv|1)] head 0
            v2 = pp.tile([128, 32 * 65], BF16)  # head 1
            attnT = pp.tile([128, T], BF16)  # [dv-part, t] normalized
            wo_g = pp.tile([128, DIM], BF16)
            ident = pp.tile([128, 128], BF16)
            identf = pp.tile([128, 128], F32)
            ones1 = pp.tile([1, 64], F32R)

            make_identity(nc, ident[:])
            make_identity(nc, identf[:])
            ones_f = pp.tile([128, 64], F32)
            nc.vector.memset(ones_f[:], 1.0)
            nc.vector.tensor_copy(ones1[:], ones_f[0:1, :])
            ones32 = pp.tile([128, 32], BF16)
            nc.vector.tensor_copy(ones32[:], ones_f[:, 0:32])
            # ones column at slot 64 of each 65-wide block of v1/v2; V
            # evacuations only write cols 0..63 of each block.
            for vv in (v1, v2):
                nc.vector.tensor_copy(
                    vv[:].rearrange("p (j c) -> p j c", c=65)[:, :, 64:65],
                    ones32[:].rearrange("p (j c) -> p j c", c=1),
                )
            # preload the exp activation table while DMAs run
            junk = pp.tile([1, 32], F32)
            nc.vector.memset(junk[:], 0.0)
            junk2 = pp.tile([1, 32], F32)
            nc.scalar.activation(junk2[:], junk[:], Exp)

            # ---------- Phase 1: QKV projection (+ V^T transpose) ----------
            with (
                tc.tile_pool(name="ph1", bufs=1) as p1,
                tc.tile_pool(name="qkv_ps", bufs=4, space="PSUM") as qkps,
            ):
                wqkv_g = p1.tile([128, 8 * 384], BF16)  # [k-part, (kt, o)]
                nc.gpsimd.dma_start(
                    wqkv_g[:].rearrange("p (kt o) -> p kt o", kt=8),
                    wqkvT_d[:].rearrange("(kt p) o -> p kt o", p=128),
                )
                nc.gpsimd.dma_start(wo_g[:], woT_d[:])
                xq = [p1.tile([128, T], BF16, name=f"xq{i}") for i in range(8)]
                vT = p1.tile([128, T], BF16)
                # quarter 0 gets a dedicated queue + fine chunks so its
                # kt-chains start as soon as each slice lands
                for th in range(2):
                    for kt in range(8):
                        c0 = th * 512
                        nc.sync.dma_start(
                            xq[kt][:, c0 : c0 + 512],
                            xT_d[kt * 128 : (kt + 1) * 128, c0 : c0 + 512],
                        )
                dmae = [nc.sync, nc.gpsimd, nc.scalar]
                n = 0
                for q in range(1, 4):
                    for kt in range(8):
                        c0 = q * 1024
                        dmae[n % 3].dma_start(
                            xq[kt][:, c0 : c0 + 1024],
                            xT_d[kt * 128 : (kt + 1) * 128, c0 : c0 + 1024],
                        )
                        n += 1

                sub = mybir.AluOpType.subtract
                for q in range(4):
                    # v first so transposes can interleave with q/k matmuls
                    for ot in (2, 1, 0):
                        for th in range(2):
                            ps = qkps.tile([128, 512], F32, tag="qk")
                            for kt in range(8):
                                nc.tensor.matmul(
                                    ps[:],
                                    wqkv_g[:, kt * 384 + ot * 128 : kt * 384 + (ot + 1) * 128],
                                    xq[kt][:, q * 1024 + th * 512 : q * 1024 + (th + 1) * 512],
                                    start=(kt == 0),
                                    stop=(kt == 7),
                                )
                            col = q * 1024 + th * 512
                            cs_ = slice(col, col + 512)
                            if ot == 2:
                                nc.vector.tensor_copy(vT[:, cs_], ps[:])
                            elif ot == 0:  # q: hi = bf16(q), lo = q - hi
                                for hh in range(2):
                                    php = ps[hh * 64 : (hh + 1) * 64, :]
                                    nc.scalar.activation(qs[hh][0:64, cs_], php, Copy)
                                    nc.vector.tensor_tensor(
                                        qs[hh][64:128, cs_], php, qs[hh][0:64, cs_], sub
                                    )
                            else:  # k: duplicated into both array halves
                                for hh in range(2):
                                    php = ps[hh * 64 : (hh + 1) * 64, :]
                                    nc.scalar.activation(ks[hh][0:64, cs_], php, Copy)
                                    nc.vector.tensor_copy(ks[hh][64:128, cs_], ks[hh][0:64, cs_])
                    for tj in range(8):  # V^T -> v1/v2 for this quarter
                        jt = q * 8 + tj
                        ptv = qkps.tile([128, 128], BF16, tag="vt", bufs=2)
                        nc.tensor.transpose(
                            ptv[:], vT[:, jt * 128 : (jt + 1) * 128], ident[:]
                        )
                        nc.vector.tensor_copy(v1[:, jt * 65 : jt * 65 + 64], ptv[:, 0:64])
                        nc.vector.tensor_copy(v2[:, jt * 65 : jt * 65 + 64], ptv[:, 64:128])

            # ---------- Phase 2: attention ----------
            with (
                tc.tile_pool(name="esb", bufs=1) as ep,
                tc.tile_pool(name="small", bufs=1) as sp,
                tc.tile_pool(name="osb", bufs=1) as osp,
                tc.tile_pool(name="s_ps", bufs=1, space="PSUM") as sps,
                tc.tile_pool(name="pv_ps", bufs=1, space="PSUM") as pvps,
            ):
                blocks = [(b, ib) for b in range(B) for ib in range(2)]

                def emit_boundary(pb, pib, step):
                    """Norm + out-projection for block (pb, pib), interleaved
                    into the next block's jt loop (or flushed at the end).
                    step 0: denominator transposes + reciprocal; 1: broadcast
                    + normalize; 2..5: two po pairs each."""
                    i0 = pb * 2048 + pib * 1024
                    key = f"{pb}_{pib}"
                    if step == 0:
                        # colsum rows -> partitions, reciprocal on 128 lanes,
                        # transpose back, DMA row-gather to [1, 1024]
                        pt = pvps.tile([128, 16], F32, tag="pv0", name=f"pt{key}")
                        for h in range(2):
                            for blk in range(8):
                                c = h * 8 + blk
                                nc.tensor.transpose(
                                    pt[:, c : c + 1],
                                    csd[key][h][0:1, blk * 128 : (blk + 1) * 128],
                                    identf[0:1, 0:1],
                                )
                        rT = sp.tile([128, 16], F32, tag="rT", name=f"rT{key}")
                        nc.vector.reciprocal(rT[:], pt[:])
                        pr = pvps.tile([16, 128], F32, tag="pv1", name=f"pr{key}")
                        nc.tensor.transpose(pr[:], rT[:], identf[:])
                        prs = sp.tile([16, 128], F32R, tag="prs", name=f"prs{key}")
                        nc.vector.tensor_copy(prs[:], pr[:])
                        r2 = [
                            sp.tile([1, 1024], F32R, tag=f"r{h}", name=f"r{key}_{h}")
                            for h in range(2)
                        ]
                        for h in range(2):
                            nc.sync.dma_start(r2[h][0:1, :], prs[h * 8 : (h + 1) * 8, :])
                        rcp[key] = r2
                    elif step == 1:
                        for h in range(2):
                            rbc = pvps.tile(
                                [64, 1024], F32, tag=f"pv{h}", name=f"rbc{key}_{h}"
                            )
                            for ih in range(2):
                                nc.tensor.matmul(
                                    rbc[:, ih * 512 : (ih + 1) * 512],
                                    ones1[:],
                                    rcp[key][h][0:1, ih * 512 : (ih + 1) * 512],
                                    start=True,
                                    stop=True,
                                )
                            rbs = sp.tile([64, 1024], F32, tag=f"rbs{h}", name=f"rbs{key}_{h}")
                            nc.vector.tensor_copy(rbs[:], rbc[:])
                            nc.vector.tensor_tensor(
                                attnT[h * 64 : (h + 1) * 64, i0 : i0 + 1024],
                                unorm[key][h][:],
                                rbs[:],
                                mult,
                            )
                    else:
                        for k in range(2):
                            tg = (step - 2) * 2 + k
                            row = i0 + tg * 128
                            po = pvps.tile(
                                [128, 1024], F32, tag=f"pv{k}", name=f"po{key}_{tg}"
                            )
                            for oh in range(2):
                                nc.tensor.matmul(
                                    po[:, oh * 512 : (oh + 1) * 512],
                                    attnT[:, row : row + 128],
                                    wo_g[:, oh * 512 : (oh + 1) * 512],
                                    start=True,
                                    stop=True,
                                )
                            ob = osp.tile(
                                [128, 1024], F32, tag="ob", bufs=6, name=f"ob{key}_{tg}"
                            )
                            # in the tail (flush) ScalarE is idle: split evacs
                            # (mid-run its strict-FIFO queue must stay clear
                            # for exps -- a waiting Copy head-of-line blocks)
                            if flush and tg % 2 == 1:
                                nc.scalar.activation(ob[:], po[:], Copy)
                            else:
                                nc.vector.tensor_copy(ob[:], po[:])
                            dmae = nc.sync if tg % 2 == 0 else nc.gpsimd
                            dmae.dma_start(out_d[row : row + 128, :], ob[:])

                unorm = {}
                rcp = {}
                csd = {}
                flush = False
                pend = []  # closures: lagged PV groups + block-end evacuations
                # drain to a per-jt target queue depth: PE slack per jt fits
                # ~1.2 PV groups, so each block's PV tail spills into the next
                # block's early key-tiles (norm at jt5-6, po pairs at jt7-8);
                # the targets keep pv allocations at jt9, after the previous
                # block's norm/po tiles in the psum rings.  The last block
                # drains aggressively so the tail chain starts early.
                TARGET = [9, 8, 7, 6, 5, 6, 7, 8, 9, 9, 9, 9, 9, 9, 9, 9]
                TARGET_LAST = [9, 8, 7, 6, 5, 6, 7, 8, 9, 9, 8, 6, 4, 2, 1, 0]
                prev = None
                for b, ib in blocks:
                    key = f"{b}_{ib}"
                    i0 = b * 2048 + ib * 1024
                    # allocated lazily at the first emit_pv so the pv-ring
                    # order is: prev block's pv -> prev's rbc/po -> ours
                    pv = []
                    e_pend = []

                    def emit_pv(jt, key=key, b=b, pv=pv, e_pend=e_pend):
                        if not pv:
                            pv.extend(
                                pvps.tile([65, 1024], F32, tag=f"pv{h}", name=f"pv{key}_{h}")
                                for h in range(2)
                            )
                        eh = e_pend.pop(0)
                        jv = (b * 16 + jt) * 65
                        for h, vv in enumerate((v1, v2)):
                            for ih in range(2):
                                nc.tensor.matmul(
                                    pv[h][:, ih * 512 : (ih + 1) * 512],
                                    vv[:, jv : jv + 65],
                                    eh[h][:, ih * 512 : (ih + 1) * 512],
                                    start=(jt == 0),
                                    stop=(jt == 15),
                                )

                    def emit_evac(key=key, pv=pv):
                        # denominator rows + unnormalized attn-out to SBUF;
                        # frees the pv psum banks (norm continues next block)
                        csd[key] = []
                        for h in range(2):
                            cs = sp.tile([1, 1024], F32, tag=f"cs{h}", name=f"cs{key}_{h}")
                            nc.vector.tensor_copy(cs[:], pv[h][64:65, :])
                            csd[key].append(cs)
                        unorm[key] = [
                            sp.tile([64, 1024], F32, tag=f"un{h}", name=f"un{key}_{h}")
                            for h in range(2)
                        ]
                        for h in range(2):
                            if flush and h == 1:  # ScalarE is idle in the tail
                                nc.scalar.activation(unorm[key][h][:], pv[h][0:64, :], Copy)
                            else:
                                nc.vector.tensor_copy(unorm[key][h][:], pv[h][0:64, :])

                    for jt in range(16):
                        j0 = b * 2048 + jt * 128
                        s_h = [
                            sps.tile([128, 1024], F32, tag=f"s{h}", name=f"s{key}_{jt}_{h}")
                            for h in range(2)
                        ]
                        e_h = [
                            ep.tile([128, 1024], BF16, tag=f"e{h}", bufs=20,
                                    name=f"e{key}_{jt}_{h}")
                            for h in range(2)
                        ]
                        # h-major so h0's S+exp only gate on ACT_h0(jt-1):
                        # the two heads' ACTs ping-pong and ScalarE stays busy
                        for h in range(2):
                            for ih in range(2):
                                nc.tensor.matmul(
                                    s_h[h][:, ih * 512 : (ih + 1) * 512],
                                    ks[h][:, j0 : j0 + 128],
                                    qs[h][:, i0 + ih * 512 : i0 + (ih + 1) * 512],
                                    start=True,
                                    stop=True,
                                )
                            nc.scalar.activation(e_h[h][:], s_h[h][:], Exp, scale=SCALE)
                        e_pend.append(e_h)
                        pend.append(lambda jt=jt, f=emit_pv: f(jt))

                        # previous block's norm + out-projection, emitted
                        # before this block's pv allocations enter the rings
                        if prev is not None:
                            if jt in (5, 6):
                                emit_boundary(prev[0], prev[1], jt - 5)
                            elif jt == 7:
                                emit_boundary(prev[0], prev[1], 2)
                                emit_boundary(prev[0], prev[1], 3)
                            elif jt == 8:
                                emit_boundary(prev[0], prev[1], 4)
                                emit_boundary(prev[0], prev[1], 5)
                        tgt = TARGET_LAST if (b, ib) == blocks[-1] else TARGET
                        while len(pend) > tgt[jt]:
                            pend.pop(0)()
                    pend.append(emit_evac)
                    prev = (b, ib)

                # drain everything and flush the last block's norm + po
                flush = True
                for f in pend:
                    f()
                pend.clear()
                for step in range(6):
                    emit_boundary(prev[0], prev[1], step)

    nc.compile()
    return nc


_NC = None


def _get_nc():
    global _NC
    if _NC is None:
        _NC = build_nc()
    return _NC


def _gate(mask):
    """Exact jax fp32 gate: sigmoid(m) > 0.5 (matches reference rounding)."""
    mask = np.asarray(mask, dtype=np.float32)
    return (np.float32(1.0) / (np.float32(1.0) + np.exp(-mask))) > np.float32(0.5)


def make_in_maps(x, qkv_weight, qkv_weight_mask, out_weight, out_weight_mask):
    import ml_dtypes

    bf16 = ml_dtypes.bfloat16
    x = np.asarray(x, dtype=np.float32)
    wqkv = np.where(_gate(qkv_weight_mask), np.asarray(qkv_weight, np.float32), 0.0)
    wo = np.where(_gate(out_weight_mask), np.asarray(out_weight, np.float32), 0.0)

    xT = np.ascontiguousarray(x.reshape(T, DIM).T).astype(bf16)
    in_maps = []
    for c in range(NCORES):
        r0 = c * DV
        sl = slice(r0, r0 + DV)
        w_shard = np.concatenate(
            [wqkv[sl], wqkv[DIM + r0 : DIM + r0 + DV], wqkv[2 * DIM + r0 : 2 * DIM + r0 + DV]],
            axis=0,
        )  # [384, 1024] rows = (q | k | v) for this core's 2 heads
        in_maps.append(
            {
                "xT": xT,
                "wqkvT": np.ascontiguousarray(w_shard.T).astype(bf16),
                "woT": np.ascontiguousarray(wo[:, sl].T).astype(bf16),
            }
        )
    return in_maps


LAST_RESULTS = None  # BassKernelResults of the most recent run (for profiling)


def kernel(
    x,
    qkv_weight,
    qkv_weight_mask,
    out_weight,
    out_weight_mask,
    out_bias,
    out_bias_mask,
    _trace=False,
    _tmpdir=None,
):
    global LAST_RESULTS
    from concourse.bass_utils import run_bass_kernel_spmd

    nc = _get_nc()
    in_maps = make_in_maps(x, qkv_weight, qkv_weight_mask, out_weight, out_weight_mask)
    res = run_bass_kernel_spmd(
        nc, in_maps, list(range(NCORES)), trace=_trace, tmpdir=_tmpdir
    )
    LAST_RESULTS = res
    out = np.zeros((T, DIM), dtype=np.float32)
    for r in res.results:
        out += r["out"]
    out_bias = np.asarray(out_bias, dtype=np.float32)
    out += np.where(_gate(out_bias_mask), out_bias, np.float32(0.0))[None, :]
    return out.reshape(B, N, DIM)
